# revision 2
# baseline (speedup 1.0000x reference)
"""OCAB (overlapping cross-attention block) Trainium2 Bass kernel.

Full inputs in, full outputs out; internally shards the B*nW window axis
across 8 NeuronCores (each core owns 2 window-rows = 32 image rows, with a
4-row halo for the overlapping k/v windows).

Pipeline per core (all matmuls bf16, fp32 accumulation):
  1. LayerNorm (norm_w/norm_b folded into projection weights on host) +
     PE-transpose to a channels-on-partitions slab with a ones row (biases
     ride the matmuls as an extra input channel).
  2. q/k projections into head-padded transposed slabs (32-row head blocks);
     v projection into a token-major slab (32-col head blocks; cols 30/31 of
     each block are 1.0 so rowsums ride the attention matmul).
  3. Per 16x16 window: S^T = k_patch^T q_patch per head (keys on partitions,
     5 patch-row chunks), exp on ScalarE (PSUM->SBUF bf16), col-packed
     attn@V accumulation, softmax-denominator broadcast via a constant
     matmul + in-place PSUM reciprocal, projection, residual add, DMA out.
"""

import os
import sys
from contextlib import ExitStack

import numpy as np
import ml_dtypes

for _p in ("/opt/trn_rl_repo", "/root/.axon_site/_ro/trn_rl_repo"):
    if os.path.isdir(_p) and _p not in sys.path:
        sys.path.append(_p)

import concourse.bass as bass
import concourse.tile as tile
from concourse import bacc, mybir
from concourse.bass_utils import run_bass_kernel_spmd

BF16 = mybir.dt.bfloat16
F32 = mybir.dt.float32
F32R = mybir.dt.float32r
bfnp = ml_dtypes.bfloat16

# ---- problem constants (hardcoded per contract) ----
C = 180
NH = 6
HD = 30
WS = 16
OWS = 24
PADW = 4
H = W = 256
EPS = 1e-5
NCORES = 8

# ---- per-core slab geometry ----
RS = 40          # slab image rows (32 + 2*4 halo)
CS = 264         # slab image cols (256 + 2*4 zero pad)
TS_REAL = RS * CS          # 10560 real slab tokens
TCH = 84                   # token chunks of 128
T = TCH * 128              # 10752 padded slab tokens
NG = 21                    # 512-token groups (21*512 == 10752)
NWIN = 32                  # windows per core (2 window-rows x 16)
CHUNK_ROWS = [5, 5, 5, 5, 4]       # patch rows per key chunk
CHUNK_KC = [r * OWS for r in CHUNK_ROWS]   # 120,120,120,120,96
# head -> column in the 4-bank S^T psum tile (same-bank pairs share row group)
ST_COL = {0: 0, 4: 256, 1: 512, 5: 768, 2: 1024, 3: 1536}
# head -> column in the packed S_exp sbuf tile
ES_COL = {0: 0, 4: 256, 1: 512, 5: 768, 2: 1024, 3: 1280}

LAST_RESULTS = None
_CACHED = None


def _build_program():
    stage = int(os.environ.get("KSTAGE", "9"))
    nc = bacc.Bacc("TRN2", target_bir_lowering=False)

    xs_d = nc.declare_dram_parameter("xs", [T, C], F32, isOutput=False)
    xr_d = nc.declare_dram_parameter("xr", [8192, C], F32, isOutput=False)
    wqk_d = nc.declare_dram_parameter("wqk", [181, 512], BF16, isOutput=False)
    wv_d = nc.declare_dram_parameter("wv", [181, 192], BF16, isOutput=False)
    wp_d = nc.declare_dram_parameter("wp", [192, C], BF16, isOutput=False)
    id_d = nc.declare_dram_parameter("ident", [128, 128], BF16, isOutput=False)
    e128_d = nc.declare_dram_parameter("e128", [128, 128], F32R, isOutput=False)
    e64_d = nc.declare_dram_parameter("e64", [64, 128], F32R, isOutput=False)
    ones_d = nc.declare_dram_parameter("ones", [1, T], BF16, isOutput=False)
    out_d = nc.declare_dram_parameter("out", [8192, C], F32, isOutput=True)

    with ExitStack() as ctx:
        tc = ctx.enter_context(tile.TileContext(nc))

        # ---- persistent slabs ----
        slab = ctx.enter_context(tc.tile_pool(name="slab", bufs=1))
        XT0 = slab.tile([128, T], BF16, tag="xt0")     # xn^T channels 0-127
        XT1 = slab.tile([53, T], BF16, tag="xt1")      # channels 128-179 + ones row 52
        QS = [
            slab.tile([128, T], BF16, tag=f"qs{i}", name=f"qs{i}") for i in range(4)
        ]
        VS = slab.tile([128, TCH * 192], BF16, tag="vs")

        wp_pool = ctx.enter_context(tc.tile_pool(name="wts", bufs=1))
        WQK0 = wp_pool.tile([128, 512], BF16, tag="wqk0")
        WQK1 = wp_pool.tile([53, 512], BF16, tag="wqk1")
        WV0 = wp_pool.tile([128, 192], BF16, tag="wv0")
        WV1 = wp_pool.tile([53, 192], BF16, tag="wv1")
        WP0 = wp_pool.tile([128, C], BF16, tag="wp0")
        WP1 = wp_pool.tile([64, C], BF16, tag="wp1")
        IDT = wp_pool.tile([128, 128], BF16, tag="id")
        E128 = wp_pool.tile([128, 128], F32R, tag="e128")
        E64 = wp_pool.tile([64, 128], F32R, tag="e64")

        nc.sync.dma_start(WQK0[:], wqk_d[0:128, :])
        nc.sync.dma_start(WQK1[:], wqk_d[128:181, :])
        nc.sync.dma_start(WV0[:], wv_d[0:128, :])
        nc.sync.dma_start(WV1[:], wv_d[128:181, :])
        nc.sync.dma_start(WP0[:], wp_d[0:128, :])
        nc.sync.dma_start(WP1[:], wp_d[128:192, :])
        nc.sync.dma_start(IDT[:], id_d[:, :])
        nc.sync.dma_start(E128[:], e128_d[:, :])
        nc.sync.dma_start(E64[:], e64_d[:, :])
        nc.sync.dma_start(XT1[52:53, :], ones_d[:, :])

        nrep = int(os.environ.get("KREPEAT", "1"))
        for rep in range(nrep):
            # ================= phase 1+2: LN, transpose, projections ============
            with ExitStack() as pctx:
                p_x = pctx.enter_context(tc.tile_pool(name="p_x", bufs=3))
                p_sm = pctx.enter_context(tc.tile_pool(name="p_sm", bufs=4))
                p_ps = pctx.enter_context(tc.tile_pool(name="p_ps", bufs=2, space="PSUM"))

                for g in range(TCH // 4):          # 21 groups of 4 token chunks
                    tp = p_ps.tile([128, 512], BF16, tag="tp")
                    tp2 = p_ps.tile([52, 512], BF16, tag="tp2")
                    for j in range(4):
                        tch = 4 * g + j
                        xt = p_x.tile([128, C], F32, tag="x")
                        nc.sync.dma_start(xt[:], xs_d[128 * tch : 128 * (tch + 1), :])
                        stats = p_sm.tile([128, 6], F32, tag="st")
                        aggr = p_sm.tile([128, 2], F32, tag="ag")
                        nc.vector.bn_stats(stats[:], xt[:])
                        nc.vector.bn_aggr(aggr[:], stats[:])
                        vpe = p_sm.tile([128, 1], F32, tag="vpe")
                        nc.gpsimd.tensor_scalar_add(vpe[:], aggr[:, 1:2], EPS)
                        sd = p_sm.tile([128, 1], F32, tag="sd")
                        nc.scalar.activation(
                            sd[:], vpe[:], mybir.ActivationFunctionType.Sqrt, bias=0.0
                        )
                        rstd = p_sm.tile([128, 1], F32, tag="rstd")
                        nc.vector.reciprocal(rstd[:], sd[:])
                        xn = p_x.tile([128, C], BF16, tag="xn")
                        nc.gpsimd.tensor_scalar(
                            xn[:],
                            xt[:],
                            aggr[:, 0:1],
                            rstd[:],
                            op0=mybir.AluOpType.subtract,
                            op1=mybir.AluOpType.mult,
                        )
                        nc.tensor.transpose(
                            tp[:, 128 * j : 128 * (j + 1)], xn[:, 0:128], IDT[:]
                        )
                        nc.tensor.transpose(
                            tp2[:, 128 * j : 128 * (j + 1)], xn[:, 128:180], IDT[:]
                        )
                    nc.vector.tensor_copy(XT0[:, 512 * g : 512 * (g + 1)], tp[:])
                    nc.vector.tensor_copy(XT1[0:52, 512 * g : 512 * (g + 1)], tp2[:])

                # q^T / k^T projections: 4 M-chunks (q03, q45, k03, k45)
                for mc in range(4):
                    for ng in range(NG):
                        qp = p_ps.tile([128, 512], F32, tag="mm")
                        nc.tensor.matmul(
                            qp[:],
                            WQK0[:, 128 * mc : 128 * (mc + 1)],
                            XT0[:, 512 * ng : 512 * (ng + 1)],
                            start=True,
                            stop=False,
                        )
                        nc.tensor.matmul(
                            qp[:],
                            WQK1[:, 128 * mc : 128 * (mc + 1)],
                            XT1[:, 512 * ng : 512 * (ng + 1)],
                            start=False,
                            stop=True,
                        )
                        nc.vector.tensor_copy(
                            QS[mc][:, 512 * ng : 512 * (ng + 1)], qp[:]
                        )

                # v projection (token-major, 32-col head blocks)
                for pair in range(TCH // 2):
                    vp = p_ps.tile([128, 384], F32, tag="vmm")
                    for j in range(2):
                        tch = 2 * pair + j
                        nc.tensor.matmul(
                            vp[:, 192 * j : 192 * (j + 1)],
                            XT0[:, 128 * tch : 128 * (tch + 1)],
                            WV0[:],
                            start=True,
                            stop=False,
                        )
                        nc.tensor.matmul(
                            vp[:, 192 * j : 192 * (j + 1)],
                            XT1[:, 128 * tch : 128 * (tch + 1)],
                            WV1[:],
                            start=False,
                            stop=True,
                        )
                    nc.vector.tensor_copy(
                        VS[:, 384 * pair : 384 * (pair + 1)], vp[:]
                    )

            # ================= phase 3: windowed attention =======================
            with ExitStack() as actx:
                a_st = actx.enter_context(tc.tile_pool(name="a_st", bufs=1, space="PSUM"))
                a_av = actx.enter_context(tc.tile_pool(name="a_av", bufs=1, space="PSUM"))
                a_ex = actx.enter_context(tc.tile_pool(name="a_ex", bufs=1, space="PSUM"))
                a_es = actx.enter_context(tc.tile_pool(name="a_es", bufs=6))
                a_vw = actx.enter_context(tc.tile_pool(name="a_vw", bufs=12))
                a_sb = actx.enter_context(tc.tile_pool(name="a_sb", bufs=2))

                qs_pat = [
                    QS[i][:, 0:TS_REAL].rearrange("p (r c) -> p r c", c=CS)
                    for i in range(4)
                ]

                if stage == 0:
                    # debug: dump VS slab into out
                    dbg = a_sb.tile([128, 360], F32, tag="ot", name=f"dbg0_{rep}")
                    nc.vector.tensor_copy(dbg[:, 0:180], VS[:, 0:180])
                    nc.sync.dma_start(out_d[0:128, :], dbg[:, 0:180])
                for w in range(NWIN if stage >= 1 else 0):
                    wrl, wc = w // 16, w % 16
                    r0, c0 = WS * wrl, WS * wc

                    # ---- gather v windows (SBUF->SBUF DMA, per patch row) ----
                    vw = [
                        a_vw.tile([128, 192], BF16, tag="vw", name=f"vw{rep}_{w}_{i}")
                        for i in range(5)
                    ]
                    for pr in range(OWS):
                        ch, rr = pr // 5, pr % 5
                        t0 = CS * (r0 + pr) + c0
                        done = 0
                        while done < OWS:
                            p0 = (t0 + done) % 128
                            blk = (t0 + done) // 128
                            n = min(OWS - done, 128 - p0)
                            nc.sync.dma_start(
                                vw[ch][OWS * rr + done : OWS * rr + done + n, :],
                                VS[p0 : p0 + n, 192 * blk : 192 * (blk + 1)],
                            )
                            done += n

                    av = a_av.tile([128, 512], F32, tag="av")

                    # materialize k^T windows (stationary matmul APs must be
                    # single-free-dim; moving APs may stay 3D)
                    kw0 = a_vw.tile([128, 576], BF16, tag="kw0", name=f"kw0_{rep}_{w}", bufs=2)
                    kw1 = a_vw.tile([64, 576], BF16, tag="kw1", name=f"kw1_{rep}_{w}", bufs=2)
                    nc.vector.tensor_copy(
                        kw0[:].rearrange("p (r c) -> p r c", c=OWS),
                        qs_pat[2][:, r0 : r0 + OWS, c0 : c0 + OWS],
                    )
                    nc.vector.tensor_copy(
                        kw1[:].rearrange("p (r c) -> p r c", c=OWS),
                        qs_pat[3][0:64, r0 : r0 + OWS, c0 : c0 + OWS],
                    )

                    if stage == 1:
                        dbg1 = a_sb.tile([128, 360], F32, tag="ot", name=f"dbg1_{rep}_{w}")
                        nc.vector.tensor_copy(dbg1[:, 0:192], vw[0][:, :])
                        nc.vector.tensor_copy(dbg1[:, 192:336], kw0[:, 0:144])
                        nc.sync.dma_start(
                            out_d[256 * w : 256 * w + 128, 0:180], dbg1[:, 0:180]
                        )
                        continue

                    if stage == 21:
                        heads = [0]
                    elif stage == 25:
                        heads = [1]
                    elif stage == 26:
                        heads = [0, 1, 2, 3]
                    elif stage == 27:
                        heads = [0, 1]
                    elif stage == 28:
                        heads = [0, 4]
                    elif stage == 22:
                        heads = [0, 1, 2]
                    elif stage == 23:
                        heads = [0, 1, 2, 3]
                    elif stage == 24:
                        heads = [0, 1, 2, 4, 5]
                    else:
                        heads = list(range(NH))
                    do_exp = stage not in (21, 22, 23, 24, 25, 26, 27, 28)
                    kdim = 32 if stage in (26,) or stage >= 9 else HD

                    es_list = []
                    for ch in range(5):
                        kc = CHUNK_KC[ch]
                        # Concurrent row-group-packed matmuls must write distinct
                        # PSUM banks: st is 4 banks; same-bank head pairs share a
                        # row group so the sub-array serializes them.
                        st = a_st.tile([128, 2048], F32, tag="st", name=f"st{rep}_{w}_{ch}")
                        for h in heads:
                            ktile = kw0 if h < 4 else kw1
                            qtile = qs_pat[0] if h < 4 else qs_pat[1]
                            hr = 32 * (h % 4)
                            kpat = ktile[hr : hr + kdim, 120 * ch : 120 * ch + kc]
                            qpat = qtile[
                                hr : hr + kdim,
                                PADW + WS * wrl : PADW + WS * wrl + WS,
                                PADW + c0 : PADW + c0 + WS,
                            ]
                            nc.tensor.matmul(
                                st[0:kc, ST_COL[h] : ST_COL[h] + 256],
                                kpat,
                                qpat,
                                start=True,
                                stop=True,
                                tile_position=(hr, 0),
                            )
                        es = a_es.tile([128, 1536], BF16, tag="es", name=f"es{rep}_{w}_{ch}")
                        if do_exp:
                            nc.scalar.activation(
                                es[0:kc, 0:1024],
                                st[0:kc, 0:1024],
                                mybir.ActivationFunctionType.Exp,
                            )
                            nc.scalar.activation(
                                es[0:kc, 1024:1536].rearrange(
                                    "p (a b) -> p a b", b=256
                                ),
                                st[0:kc, 1024:2048].rearrange("p (a b) -> p a b", b=512)[
                                    :, :, 0:256
                                ],
                                mybir.ActivationFunctionType.Exp,
                            )
                        else:
                            nc.vector.tensor_copy(
                                es[0:kc, 0:256], st[0:kc, 0:256]
                            )
                        es_list.append(es)

                    if stage in (2, 21, 22, 23, 24, 25, 26, 27, 28):
                        dbg2 = a_sb.tile([128, 360], F32, tag="ot", name=f"dbg2_{rep}_{w}")
                        nc.vector.tensor_copy(dbg2[:, 0:256], es_list[0][:, 0:256])
                        nc.sync.dma_start(
                            out_d[256 * w : 256 * w + 128, 0:180], dbg2[:, 0:180]
                        )
                        continue

                    # ---- attn @ V (col-packed; h4/h5 duplicated to fill psum) ----
                    # head-major so psum accumulation groups are sequential
                    av_jobs = [(h, 32 * (h % 4), 256 * (h // 4)) for h in range(NH)]
                    av_jobs += [(h, 64 + 32 * (h - 4), 256) for h in (4, 5)]
                    for h, colp, colf in av_jobs:
                        for ch in range(5):
                            kc = CHUNK_KC[ch]
                            nc.tensor.matmul(
                                av[colp : colp + 32, colf : colf + 256],
                                vw[ch][0:kc, 32 * h : 32 * h + 32],
                                es_list[ch][0:kc, ES_COL[h] : ES_COL[h] + 256],
                                start=(ch == 0),
                                stop=(ch == 4),
                                tile_position=(0, colp),
                            )

                    if stage == 3:
                        dbg3 = a_sb.tile([128, 360], F32, tag="ot", name=f"dbg3_{rep}_{w}")
                        nc.vector.tensor_copy(dbg3[:, 0:256], av[:, 0:256])
                        nc.sync.dma_start(
                            out_d[256 * w : 256 * w + 128, 0:180], dbg3[:, 0:180]
                        )
                        continue

                    # ---- softmax normalize + project + residual ----
                    rsb = a_sb.tile([128, 512], F32R, tag="rsb")
                    nc.vector.tensor_copy(rsb[:], av[:])
                    ex = a_ex.tile([128, 512], F32, tag="ex")
                    nc.tensor.matmul(
                        ex[:, 0:256],
                        E128[:],
                        rsb[:, 0:256],
                        start=True,
                        stop=True,
                    )
                    nc.tensor.matmul(
                        ex[:, 256:512],
                        E64[:],
                        rsb[0:64, 256:512],
                        start=True,
                        stop=True,
                    )
                    ex_sb = a_sb.tile([128, 512], F32, tag="exsb")
                    nc.vector.reciprocal(ex_sb[:], ex[:])
                    att = a_sb.tile([128, 512], BF16, tag="att")
                    nc.vector.tensor_tensor(
                        att[:], av[:], ex_sb[:], op=mybir.AluOpType.mult
                    )

                    pp = a_ex.tile([128, 360], F32, tag="pp", name=f"pp{rep}_{w}")
                    for qc in range(2):
                        nc.tensor.matmul(
                            pp[:, 180 * qc : 180 * qc + 180],
                            att[:, 128 * qc : 128 * (qc + 1)],
                            WP0[:],
                            start=True,
                            stop=False,
                        )
                        nc.tensor.matmul(
                            pp[:, 180 * qc : 180 * qc + 180],
                            att[0:64, 256 + 128 * qc : 256 + 128 * (qc + 1)],
                            WP1[:],
                            start=False,
                            stop=True,
                        )
                    xres = a_sb.tile([128, 360], F32, tag="xres")
                    ot = a_sb.tile([128, 360], F32, tag="ot")
                    xr_pat = xr_d[:, :].rearrange("(r c) d -> r c d", c=W)
                    for qc in range(2):
                        rq = WS * wrl + 8 * qc
                        nc.sync.dma_start(
                            xres[:, 180 * qc : 180 * qc + 180],
                            xr_pat[rq : rq + 8, c0 : c0 + WS, :],
                        )
                    nc.vector.tensor_tensor(
                        ot[:], pp[:], xres[:], op=mybir.AluOpType.add
                    )
                    out_pat = out_d[:, :].rearrange("(r c) d -> r c d", c=W)
                    for qc in range(2):
                        rq = WS * wrl + 8 * qc
                        nc.sync.dma_start(
                            out_pat[rq : rq + 8, c0 : c0 + WS, :],
                            ot[:, 180 * qc : 180 * qc + 180],
                        )

    nc.compile()
    return nc


def _prep_host(inputs):
    x = np.ascontiguousarray(inputs["x"], dtype=np.float32)[0]  # [65536, 180]
    norm_w = np.asarray(inputs["norm_w"], np.float32)
    norm_b = np.asarray(inputs["norm_b"], np.float32)
    q_w = np.asarray(inputs["q_w"], np.float32)
    q_b = np.asarray(inputs["q_b"], np.float32)
    kv_w = np.asarray(inputs["kv_w"], np.float32)
    kv_b = np.asarray(inputs["kv_b"], np.float32)
    proj_w = np.asarray(inputs["proj_w"], np.float32)
    proj_b = np.asarray(inputs["proj_b"], np.float32)

    scale = HD ** -0.5
    Wq = norm_w[:, None] * q_w * scale
    bq = (norm_b @ q_w + q_b) * scale
    Wk = norm_w[:, None] * kv_w[:, :C]
    bk = norm_b @ kv_w[:, :C] + kv_b[:C]
    Wv = norm_w[:, None] * kv_w[:, C:]
    bv = norm_b @ kv_w[:, C:] + kv_b[C:]

    # wqk [181, 512]: 4 M-chunks (q h0-3 | q h4-5 | k h0-3 | k h4-5), 32-col head blocks
    wqk = np.zeros((181, 512), np.float32)
    for h in range(NH):
        mc = 0 if h < 4 else 1
        col = 128 * mc + 32 * (h % 4)
        wqk[:C, col : col + HD] = Wq[:, HD * h : HD * (h + 1)]
        wqk[C, col : col + HD] = bq[HD * h : HD * (h + 1)]
        colk = 256 + col
        wqk[:C, colk : colk + HD] = Wk[:, HD * h : HD * (h + 1)]
        wqk[C, colk : colk + HD] = bk[HD * h : HD * (h + 1)]

    # wv [181, 192]: 32-col head blocks; cols 30/31 of each block = ones (bias row)
    wv = np.zeros((181, 192), np.float32)
    for h in range(NH):
        col = 32 * h
        wv[:C, col : col + HD] = Wv[:, HD * h : HD * (h + 1)]
        wv[C, col : col + HD] = bv[HD * h : HD * (h + 1)]
        wv[C, col + 30] = 1.0
        wv[C, col + 31] = 1.0

    # wp [192, 180]: head-padded proj rows
    wp = np.zeros((192, C), np.float32)
    for h in range(NH):
        row = 32 * (h % 4) if h < 4 else 128 + 32 * (h - 4)
        wp[row : row + HD, :] = proj_w[HD * h : HD * (h + 1), :]

    ident = np.eye(128, dtype=bfnp)
    e128 = np.zeros((128, 128), np.float32)
    for j in range(4):
        e128[32 * j + 30, 32 * j : 32 * j + 32] = 1.0
    e64 = np.zeros((64, 128), np.float32)
    for j in range(2):
        e64[32 * j + 30, 32 * j : 32 * j + 32] = 1.0
        e64[32 * j + 30, 64 + 32 * j : 64 + 32 * j + 32] = 1.0

    # per-core slabs
    xg = x.reshape(H, W, C)
    xpad = np.zeros((H + 2 * PADW, CS, C), np.float32)
    xpad[PADW : PADW + H, PADW : PADW + W, :] = xg
    xres_full = x + proj_b  # residual (+proj bias folded in)

    in_maps = []
    for c in range(NCORES):
        slab = np.zeros((T, C), np.float32)
        slab[:TS_REAL] = xpad[32 * c : 32 * c + RS].reshape(TS_REAL, C)
        xr = np.ascontiguousarray(
            xres_full[8192 * c : 8192 * (c + 1)], dtype=np.float32
        )
        in_maps.append(
            {
                "xs": slab,
                "xr": xr,
                "wqk": wqk.astype(bfnp),
                "wv": wv.astype(bfnp),
                "wp": wp.astype(bfnp),
                "ident": ident,
                "e128": e128,
                "e64": e64,
                "ones": np.ones((1, T), bfnp),
            }
        )
    return in_maps


def kernel(**inputs):
    global _CACHED, LAST_RESULTS
    if _CACHED is None:
        _CACHED = _build_program()
    nc = _CACHED
    in_maps = _prep_host(inputs)
    res = run_bass_kernel_spmd(
        nc,
        in_maps,
        list(range(NCORES)),
        trace=bool(int(os.environ.get("KTRACE", "0"))),
    )
    LAST_RESULTS = res
    out = np.empty((1, H * W, C), np.float32)
    for c in range(NCORES):
        out[0, 8192 * c : 8192 * (c + 1), :] = res.results[c]["out"]
    return out



# revision 17
# speedup vs baseline: 1.4432x; 1.4432x over previous
"""OCAB (overlapping cross-attention block) Trainium2 Bass kernel, v2.

Full inputs in, full outputs out; internally shards the B*nW window axis
across 8 NeuronCores (each core owns 2 window-rows = 32 image rows, with a
4-row halo for the overlapping k/v windows).

Per core:
  Phase 1+2 (fused, 21 groups of 512 tokens): stream x, LayerNorm
  (stats on vector, normalize on scalar Identity act with per-partition
  scale/bias), PE-transpose to channel-major group tiles (ones channel at
  row 180 rides the transposes), then 5 projection passes per group into
  persistent transposed slabs: q h0-3, k h0-3, v h0-3, (q45|k45) stacked,
  v h4-5. Biases ride as an extra input channel; LN gamma/beta and the
  attention scale are folded into the weights on host.

  Phase 3 (32 windows, software-pipelined): per window materialize
  contiguous k^T/v^T windows (vector copies), build token-major v tiles
  with DMA transposes, compute S^T per head with row-packed matmuls into
  a double-buffered 3-bank PSUM ring, exp on scalar (one act per key
  chunk), chunk-major attn@V accumulation (denominators ride as ones
  columns of v), denominator broadcast via constant matmuls, fast approx
  reciprocal, projection, residual add, contiguous window-major DRAM IO
  (host pre/post-permutes the window order).
"""

import os
import sys
from contextlib import ExitStack

import numpy as np
import ml_dtypes

for _p in ("/opt/trn_rl_repo", "/root/.axon_site/_ro/trn_rl_repo"):
    if os.path.isdir(_p) and _p not in sys.path:
        sys.path.append(_p)

import concourse.bass as bass
import concourse.tile as tile
from concourse import bacc, mybir
from concourse.bass_utils import run_bass_kernel_spmd

BF16 = mybir.dt.bfloat16
F32 = mybir.dt.float32
F32R = mybir.dt.float32r
bfnp = ml_dtypes.bfloat16
AF = mybir.ActivationFunctionType
ALU = mybir.AluOpType

# ---- problem constants (hardcoded per contract) ----
C = 180
NH = 6
HD = 30
WS = 16
OWS = 24
PADW = 4
H = W = 256
EPS = 1e-5
NCORES = 8

# ---- per-core slab geometry ----
RS = 40                    # slab image rows (32 + 2*4 halo)
CS = 264                   # slab image cols (256 + 2*4 zero pad)
TS_REAL = RS * CS          # 10560 real slab tokens
TCH = 84                   # token chunks of 128
T = TCH * 128              # 10752 padded slab tokens
NG = 21                    # 512-token groups
NWIN = 32                  # windows per core (2 window-rows x 16)

KOFF = [0, 128, 256, 384, 512]     # key chunk offsets
KC = [128, 128, 128, 128, 64]      # keys per chunk
# head -> column in es sbuf
EC = {0: 0, 4: 256, 1: 512, 5: 768, 2: 1024, 3: 1280}
# head -> column in S^T psum: h0/h4/h1/h5 in st_a, h2/h3 in st_b
STC = {0: 0, 4: 256, 1: 512, 5: 768, 2: 0, 3: 512}
# head -> PE row group (must equal stationary base partition)
TPR = {0: 0, 1: 32, 2: 64, 3: 96, 4: 0, 5: 32}
STW = [[0, 1, 2], [4, 5, 3]]       # S^T emission waves (distinct banks)
# head -> column block in token-major vw tiles
VC = {0: 0, 1: 32, 2: 64, 3: 96, 4: 128, 5: 160}
# attn@V jobs: (head, av row offset, av col offset)
AVJ = [(0, 0, 0), (1, 32, 0), (2, 64, 0), (3, 96, 0), (4, 0, 256), (5, 32, 256)]

LAST_RESULTS = None
_CACHED = None


def _build_program():
    nc = bacc.Bacc("TRN2", target_bir_lowering=False)

    xs_d = nc.declare_dram_parameter("xs", [T, C], F32, isOutput=False)
    xrw_d = nc.declare_dram_parameter("xrw", [8192, C], F32, isOutput=False)
    wqkv_d = nc.declare_dram_parameter("wqkv", [181, 576], BF16, isOutput=False)
    wp_d = nc.declare_dram_parameter("wp", [192, C], BF16, isOutput=False)
    id_d = nc.declare_dram_parameter("ident", [128, 128], BF16, isOutput=False)
    e128_d = nc.declare_dram_parameter("e128", [128, 128], F32R, isOutput=False)
    e64_d = nc.declare_dram_parameter("e64", [64, 64], F32R, isOutput=False)
    out_d = nc.declare_dram_parameter("out", [8192, C], F32, isOutput=True)
    dbg_d = nc.declare_dram_parameter("dbg", [128, 2048], F32, isOutput=True)
    kstage = os.environ.get("KSTAGE", "")

    with ExitStack() as ctx:
        tc = ctx.enter_context(tile.TileContext(nc))

        wp_pool = ctx.enter_context(tc.tile_pool(name="wts", bufs=1))
        WQKV0 = wp_pool.tile([128, 576], BF16, tag="wqkv0")
        WQKV1 = wp_pool.tile([53, 576], BF16, tag="wqkv1")
        WP0 = wp_pool.tile([128, C], BF16, tag="wp0")
        WP1 = wp_pool.tile([64, C], BF16, tag="wp1")
        IDT = wp_pool.tile([128, 128], BF16, tag="id")
        E128 = wp_pool.tile([128, 128], F32R, tag="e128")
        E64 = wp_pool.tile([64, 64], F32R, tag="e64")

        nc.sync.dma_start(WQKV0[:], wqkv_d[0:128, :])
        nc.sync.dma_start(WQKV1[:], wqkv_d[128:181, :])
        nc.sync.dma_start(WP0[:], wp_d[0:128, :])
        nc.sync.dma_start(WP1[:], wp_d[128:192, :])
        nc.sync.dma_start(IDT[:], id_d[:, :])
        nc.sync.dma_start(E128[:], e128_d[:, :])
        nc.sync.dma_start(E64[:], e64_d[:, :])

        # persistent transposed slabs
        slab = ctx.enter_context(tc.tile_pool(name="slab", bufs=1))
        Q03 = slab.tile([128, T], BF16, tag="q03")
        K03 = slab.tile([128, T], BF16, tag="k03")
        V03 = slab.tile([128, T], BF16, tag="v03")
        QK45 = slab.tile([128, T], BF16, tag="qk45")   # rows 0:64 q45, 64:128 k45
        V45 = slab.tile([64, T], BF16, tag="v45")
        SLABS = [Q03, K03, V03, QK45, V45]
        SLAB_ROWS = [128, 128, 128, 128, 64]
        # which engine copies each pass's psum to its slab
        COPY_ENG = ["scalar", "scalar", "scalar", "vector", "vector"]

        # ================= phase 1+2: LN + transpose + projections ==========
        with ExitStack() as pctx:
            p_x = pctx.enter_context(tc.tile_pool(name="p_x", bufs=4))
            p_sm = pctx.enter_context(tc.tile_pool(name="p_sm", bufs=4))
            p_tp = pctx.enter_context(tc.tile_pool(name="p_tp", bufs=2, space="PSUM"))
            p_qp = pctx.enter_context(tc.tile_pool(name="p_qp", bufs=3, space="PSUM"))
            p_xtg = pctx.enter_context(tc.tile_pool(name="p_xtg", bufs=3))

            for g in range(NG):
                tp = p_tp.tile([128, 512], BF16, tag="tp")
                tp2 = p_tp.tile([128, 512], BF16, tag="tp2")
                for j in range(4):
                    tch = 4 * g + j
                    xt = p_x.tile([128, C], F32, tag="x")
                    nc.sync.dma_start(xt[:], xs_d[128 * tch : 128 * (tch + 1), :])
                    stats = p_sm.tile([128, 6], F32, tag="st")
                    aggr = p_sm.tile([128, 2], F32, tag="ag")
                    nc.vector.bn_stats(stats[:], xt[:])
                    nc.vector.bn_aggr(aggr[:], stats[:])
                    vpe = p_sm.tile([128, 1], F32, tag="vpe")
                    nc.gpsimd.tensor_scalar_add(vpe[:], aggr[:, 1:2], EPS)
                    sd = p_sm.tile([128, 1], F32, tag="sd")
                    nc.scalar.activation(sd[:], vpe[:], AF.Sqrt, bias=0.0)
                    rstd = p_sm.tile([128, 1], F32, tag="rstd")
                    nc.vector.reciprocal(rstd[:], sd[:])
                    xn = p_x.tile([128, 256], BF16, tag="xn")
                    if int(os.environ.get("KLNV", "1")):
                        nc.vector.tensor_scalar(
                            xn[:, 0:C], xt[:], aggr[:, 0:1], rstd[:],
                            op0=ALU.subtract, op1=ALU.mult,
                        )
                    else:
                        nbias = p_sm.tile([128, 1], F32, tag="nb")
                        nc.vector.scalar_tensor_tensor(
                            nbias[:], aggr[:, 0:1], -1.0, rstd[:],
                            op0=ALU.mult, op1=ALU.mult,
                        )
                        nc.scalar.activation(
                            xn[:, 0:C], xt[:], AF.Identity,
                            bias=nbias[:], scale=rstd[:],
                        )
                    nc.gpsimd.memset(xn[:, C : C + 1], 1.0)
                    nc.tensor.transpose(
                        tp[:, 128 * j : 128 * (j + 1)], xn[:, 0:128], IDT[:]
                    )
                    nc.tensor.transpose(
                        tp2[:, 128 * j : 128 * (j + 1)], xn[:, 128:256], IDT[:]
                    )
                xt0g = p_xtg.tile([128, 512], BF16, tag="xt0g")
                xt1g = p_xtg.tile([128, 512], BF16, tag="xt1g")
                nc.vector.tensor_copy(xt0g[:], tp[:])
                nc.vector.tensor_copy(xt1g[0:53, :], tp2[0:53, :])

                for p in range(5):
                    c0, c1 = (128 * p, 128 * p + 128) if p < 4 else (512, 576)
                    outw = SLAB_ROWS[p]
                    qp = p_qp.tile([128, 512], F32, tag="qp")
                    nc.tensor.matmul(
                        qp[0:outw, :], WQKV0[:, c0:c1], xt0g[:],
                        start=True, stop=False,
                    )
                    nc.tensor.matmul(
                        qp[0:outw, :], WQKV1[:, c0:c1], xt1g[0:53, :],
                        start=False, stop=True,
                    )
                    dst = SLABS[p][0:outw, 512 * g : 512 * (g + 1)]
                    if COPY_ENG[p] == "scalar":
                        nc.scalar.copy(dst, qp[0:outw, :])
                    else:
                        nc.vector.tensor_copy(dst, qp[0:outw, :])

        # ================= phase 3: windowed attention =======================
        with ExitStack() as actx:
            a_sta = actx.enter_context(tc.tile_pool(name="a_sta", bufs=2, space="PSUM"))
            a_stb = actx.enter_context(tc.tile_pool(name="a_stb", bufs=1, space="PSUM"))
            a_ep = actx.enter_context(tc.tile_pool(name="a_ep", bufs=1, space="PSUM"))
            a_avp = actx.enter_context(tc.tile_pool(name="a_avp", bufs=1, space="PSUM"))
            a_kv = actx.enter_context(tc.tile_pool(name="a_kv", bufs=2))
            a_vw = actx.enter_context(tc.tile_pool(name="a_vw", bufs=2))
            a_es = actx.enter_context(tc.tile_pool(name="a_es", bufs=7))
            a_sb = actx.enter_context(tc.tile_pool(name="a_sb", bufs=2))

            q03_pat = Q03[:, 0:TS_REAL].rearrange("p (r c) -> p r c", c=CS)
            k03_pat = K03[:, 0:TS_REAL].rearrange("p (r c) -> p r c", c=CS)
            v03_pat = V03[:, 0:TS_REAL].rearrange("p (r c) -> p r c", c=CS)
            qk45_pat = QK45[:, 0:TS_REAL].rearrange("p (r c) -> p r c", c=CS)
            v45_pat = V45[:, 0:TS_REAL].rearrange("p (r c) -> p r c", c=CS)

            # per-window state carried across pipeline stages
            st_prev = {}

            for w in range(NWIN + 1):
                cur = w < NWIN
                prev = w > 0
                if cur:
                    wrl, wc = w // 16, w % 16
                    r0, c0 = WS * wrl, WS * wc

                # ---- 1. denominator broadcast for window w-1 ----
                if prev:
                    P = st_prev
                    ex = a_ep.tile([128, 512], F32, tag="ep", name=f"ex{w}")
                    nc.tensor.matmul(
                        ex[:, 0:256], E128[:], P["rsb"][:, 0:256],
                        start=True, stop=True,
                    )
                    nc.tensor.matmul(
                        ex[0:64, 256:512], E64[:], P["rsb"][0:64, 256:512],
                        start=True, stop=True,
                    )

                # ---- 2. k^T / v^T window materialization (vector) ----
                if cur:
                    kw0 = a_kv.tile([128, 576], BF16, tag="kw0", name=f"kw0_{w}")
                    kw1 = a_kv.tile([64, 576], BF16, tag="kw1", name=f"kw1_{w}")
                    vwT0 = a_kv.tile([128, 640], BF16, tag="vwT0", name=f"vwT0_{w}")
                    vwT1 = a_kv.tile([64, 640], BF16, tag="vwT1", name=f"vwT1_{w}")
                    nc.vector.tensor_copy(
                        kw0[:].rearrange("p (r c) -> p r c", c=OWS),
                        k03_pat[:, r0 : r0 + OWS, c0 : c0 + OWS],
                    )
                    nc.vector.tensor_copy(
                        kw1[:].rearrange("p (r c) -> p r c", c=OWS),
                        qk45_pat[64:128, r0 : r0 + OWS, c0 : c0 + OWS],
                    )
                    nc.vector.tensor_copy(
                        vwT0[:, 0:576].rearrange("p (r c) -> p r c", c=OWS),
                        v03_pat[:, r0 : r0 + OWS, c0 : c0 + OWS],
                    )
                    nc.gpsimd.memset(vwT0[:, 576:640], 0.0)
                    nc.vector.tensor_copy(
                        vwT1[:, 0:576].rearrange("p (r c) -> p r c", c=OWS),
                        v45_pat[:, r0 : r0 + OWS, c0 : c0 + OWS],
                    )
                    nc.gpsimd.memset(vwT1[:, 576:640], 0.0)

                # ---- 3. token-major v tiles via DMA transpose; xres load ----
                if cur:
                    vw = [
                        a_vw.tile([128, 192], BF16, tag=f"vw{i}", name=f"vw{w}_{i}")
                        for i in range(5)
                    ]
                    for i, off in enumerate(KOFF):
                        nc.sync.dma_start_transpose(
                            vw[i][:, 0:128], vwT0[:, off : off + 128]
                        )
                        nc.sync.dma_start_transpose(
                            vw[i][:, 128:192], vwT1[:, off : off + 128]
                        )
                if prev:
                    P = st_prev
                    xres = a_sb.tile([128, 360], F32, tag="xres", name=f"xres{w}")
                    nc.sync.dma_start(
                        xres[:].rearrange("p (j d) -> p j d", d=C),
                        xrw_d[256 * (w - 1) : 256 * w, :].rearrange(
                            "(j p) d -> p j d", p=128
                        ),
                    )

                # ---- 4. normalize window w-1 (vector) ----
                if prev:
                    P = st_prev
                    exsb = a_sb.tile([128, 512], F32, tag="exsb", name=f"exsb{w}")
                    if int(os.environ.get("KRECIP", "1")):
                        nc.vector.reciprocal_approx_fast(exsb[:, 0:256], ex[:, 0:256])
                        nc.vector.reciprocal_approx_fast(
                            exsb[0:64, 256:512], ex[0:64, 256:512]
                        )
                    else:
                        nc.vector.reciprocal(exsb[:, 0:256], ex[:, 0:256])
                        nc.vector.reciprocal(exsb[0:64, 256:512], ex[0:64, 256:512])
                    att = a_sb.tile([128, 512], BF16, tag="att", name=f"att{w}")
                    nc.vector.tensor_tensor(
                        att[:, 0:256], P["av"][:, 0:256], exsb[:, 0:256],
                        op=ALU.mult,
                    )
                    nc.vector.tensor_tensor(
                        att[0:64, 256:512], P["av"][0:64, 256:512],
                        exsb[0:64, 256:512], op=ALU.mult,
                    )

                # ---- 5. S^T + exp + attn@V for window w ----
                if cur:
                    av = a_avp.tile([128, 512], F32, tag="av", name=f"av{w}")
                    es_list = []

                    def st_chunk(ch):
                        off, kc = KOFF[ch], KC[ch]
                        sta = a_sta.tile(
                            [128, 1024], F32, tag="sta", name=f"sta{w}_{ch}"
                        )
                        stb = a_stb.tile(
                            [128, 1024], F32, tag="stb", name=f"stb{w}_{ch}"
                        )
                        for wave in STW:
                            for h in wave:
                                if h < 4:
                                    ktile, qtile = kw0, q03_pat
                                    kr, qr = 32 * h, 32 * h
                                else:
                                    ktile, qtile = kw1, qk45_pat
                                    kr, qr = 32 * (h - 4), 32 * (h - 4)
                                st = sta if h in (0, 1, 4, 5) else stb
                                nc.tensor.matmul(
                                    st[0:kc, STC[h] : STC[h] + 256],
                                    ktile[kr : kr + 32, off : off + kc],
                                    qtile[
                                        qr : qr + 32,
                                        PADW + r0 : PADW + r0 + WS,
                                        PADW + c0 : PADW + c0 + WS,
                                    ],
                                    start=True, stop=True,
                                    tile_position=(TPR[h], 0),
                                )
                        return sta, stb

                    def exp_chunk(ch, sta, stb):
                        kc = KC[ch]
                        es = a_es.tile(
                            [128, 1536], BF16, tag="es", name=f"es{w}_{ch}"
                        )
                        nc.scalar.activation(
                            es[0:kc, 0:1024], sta[0:kc, :], AF.Exp
                        )
                        nc.scalar.activation(
                            es[0:kc, 1024:1536].rearrange(
                                "p (a b) -> p a b", b=256
                            ),
                            stb[0:kc, :].rearrange("p (a b) -> p a b", b=512)[
                                :, :, 0:256
                            ],
                            AF.Exp,
                        )
                        es_list.append(es)

                    def av_one(h, ro, co, ch):
                        # jobs with disjoint psum partitions may hold
                        # concurrently-open accumulation groups; jobs sharing
                        # partitions+bank (h0/h4, h1/h5) must not interleave
                        kc = KC[ch]
                        nc.tensor.matmul(
                            av[ro : ro + 32, co : co + 256],
                            vw[ch][0:kc, VC[h] : VC[h] + 32],
                            es_list[ch][0:kc, EC[h] : EC[h] + 256],
                            start=(ch == 0), stop=(ch == 4),
                            tile_position=(0, ro),
                            skip_group_check=True,
                        )

                    def av_chunk(ch):
                        for h, ro, co in AVJ[:4]:
                            av_one(h, ro, co, ch)

                    sts = [st_chunk(0)]
                    exp_chunk(0, *sts[0])
                    sts.append(st_chunk(1))
                    exp_chunk(1, *sts[1])
                    av_chunk(0)
                    sts.append(st_chunk(2))
                    exp_chunk(2, *sts[2])
                    av_chunk(1)
                    sts.append(st_chunk(3))
                    exp_chunk(3, *sts[3])
                    av_chunk(2)
                    sts.append(st_chunk(4))
                    exp_chunk(4, *sts[4])
                    av_chunk(3)
                    av_chunk(4)
                    for ch in range(5):
                        for h, ro, co in AVJ[4:]:
                            av_one(h, ro, co, ch)

                # ---- 6. projection + residual + store for window w-1 ----
                if prev:
                    P = st_prev
                    pp = a_ep.tile([128, 360], F32, tag="ep", name=f"pp{w}")
                    for qc in range(2):
                        nc.tensor.matmul(
                            pp[:, 180 * qc : 180 * qc + 180],
                            att[:, 128 * qc : 128 * (qc + 1)],
                            WP0[:],
                            start=True, stop=False,
                        )
                        nc.tensor.matmul(
                            pp[:, 180 * qc : 180 * qc + 180],
                            att[0:64, 256 + 128 * qc : 256 + 128 * (qc + 1)],
                            WP1[:],
                            start=False, stop=True,
                        )
                    ot = a_sb.tile([128, 360], F32, tag="ot", name=f"ot{w}")
                    nc.vector.tensor_tensor(ot[:], pp[:], xres[:], op=ALU.add)
                    nc.sync.dma_start(
                        out_d[256 * (w - 1) : 256 * w, :].rearrange(
                            "(j p) d -> p j d", p=128
                        ),
                        ot[:].rearrange("p (j d) -> p j d", d=C),
                    )

                # ---- debug dumps for window 0 ----
                if cur and w == 0 and kstage:
                    dbg = a_sb.tile([128, 2048], F32, tag="dbg", bufs=1)
                    if kstage == "q":
                        nc.vector.tensor_copy(dbg[:, 0:2048], Q03[:, 0:2048])
                    elif kstage == "kw":
                        nc.vector.tensor_copy(dbg[:, 0:576], kw0[:])
                        nc.vector.tensor_copy(dbg[0:64, 576:1152], kw1[:])
                    elif kstage == "vw0":
                        nc.vector.tensor_copy(dbg[:, 0:192], vw[0][:])
                        nc.vector.tensor_copy(dbg[:, 192:384], vw[4][:])
                    elif kstage == "es0":
                        nc.vector.tensor_copy(dbg[:, 0:1536], es_list[0][:])
                    elif kstage == "es4":
                        nc.vector.tensor_copy(dbg[0:64, 0:1536], es_list[4][0:64, :])
                    elif kstage == "av":
                        nc.vector.tensor_copy(dbg[:, 0:512], av[:])
                    nc.sync.dma_start(dbg_d[:, :], dbg[:])

                # ---- 7. rowsum snapshot for window w ----
                if cur:
                    rsb = a_sb.tile([128, 512], F32R, tag="rsb", name=f"rsb{w}")
                    nc.vector.tensor_copy(rsb[:], av[:])
                    st_prev = {"av": av, "rsb": rsb}

    nc.compile()
    return nc


def _prep_host(inputs):
    x = np.ascontiguousarray(inputs["x"], dtype=np.float32)[0]  # [65536, 180]
    norm_w = np.asarray(inputs["norm_w"], np.float32)
    norm_b = np.asarray(inputs["norm_b"], np.float32)
    q_w = np.asarray(inputs["q_w"], np.float32)
    q_b = np.asarray(inputs["q_b"], np.float32)
    kv_w = np.asarray(inputs["kv_w"], np.float32)
    kv_b = np.asarray(inputs["kv_b"], np.float32)
    proj_w = np.asarray(inputs["proj_w"], np.float32)
    proj_b = np.asarray(inputs["proj_b"], np.float32)

    scale = HD ** -0.5
    Wq = norm_w[:, None] * q_w * scale
    bq = (norm_b @ q_w + q_b) * scale
    Wk = norm_w[:, None] * kv_w[:, :C]
    bk = norm_b @ kv_w[:, :C] + kv_b[:C]
    Wv = norm_w[:, None] * kv_w[:, C:]
    bv = norm_b @ kv_w[:, C:] + kv_b[C:]

    # wqkv [181, 576]: q03 | k03 | v03 | (q45|k45) | v45, 32-col head blocks;
    # v blocks carry 1.0 at row 180 in cols 30/31 (denominator ride-along)
    wqkv = np.zeros((181, 576), np.float32)

    def put(colbase, h, Wm, bm, ones):
        col = colbase + 32 * (h % 4)
        wqkv[:C, col : col + HD] = Wm[:, HD * h : HD * (h + 1)]
        wqkv[C, col : col + HD] = bm[HD * h : HD * (h + 1)]
        if ones:
            wqkv[C, col + 30] = 1.0
            wqkv[C, col + 31] = 1.0

    for h in range(4):
        put(0, h, Wq, bq, False)
        put(128, h, Wk, bk, False)
        put(256, h, Wv, bv, True)
    for h in (4, 5):
        put(384, h, Wq, bq, False)
        put(448, h, Wk, bk, False)
        put(512, h, Wv, bv, True)

    # wp [192, 180]: rows 0:128 = proj rows h0-3 (32-blocks), 128:192 h4-5
    wp = np.zeros((192, C), np.float32)
    for h in range(NH):
        row = 32 * h if h < 4 else 128 + 32 * (h - 4)
        wp[row : row + HD, :] = proj_w[HD * h : HD * (h + 1), :]

    ident = np.eye(128, dtype=bfnp)
    e128 = np.zeros((128, 128), np.float32)
    for j in range(4):
        e128[32 * j + 30, 32 * j : 32 * j + 32] = 1.0
    e64 = np.zeros((64, 64), np.float32)
    e64[30, 0:32] = 1.0
    e64[62, 32:64] = 1.0

    # per-core slabs
    xg = x.reshape(H, W, C)
    xpad = np.zeros((H + 2 * PADW, CS, C), np.float32)
    xpad[PADW : PADW + H, PADW : PADW + W, :] = xg
    xres_full = (x + proj_b).reshape(H, W, C)

    in_maps = []
    for c in range(NCORES):
        slab_x = np.zeros((T, C), np.float32)
        slab_x[:TS_REAL] = xpad[32 * c : 32 * c + RS].reshape(TS_REAL, C)
        # window-major residual: [2 wrl, 16 r, 16 wc, 16 cc, C] ->
        # (wrl, wc, r, cc)
        xr = xres_full[32 * c : 32 * c + 32].reshape(2, 16, 16, 16, C)
        xr = np.ascontiguousarray(xr.transpose(0, 2, 1, 3, 4)).reshape(8192, C)
        in_maps.append(
            {
                "xs": slab_x,
                "xrw": xr,
                "wqkv": wqkv.astype(bfnp),
                "wp": wp.astype(bfnp),
                "ident": ident,
                "e128": e128,
                "e64": e64,
            }
        )
    return in_maps


def kernel(**inputs):
    global _CACHED, LAST_RESULTS
    if _CACHED is None:
        _CACHED = _build_program()
    nc = _CACHED
    in_maps = _prep_host(inputs)
    res = run_bass_kernel_spmd(
        nc,
        in_maps,
        list(range(NCORES)),
        trace=bool(int(os.environ.get("KTRACE", "0"))),
    )
    LAST_RESULTS = res
    out = np.empty((1, H * W, C), np.float32)
    og = out[0].reshape(H, W, C)
    for c in range(NCORES):
        # un-permute window-major [2, 16 wc, 16 r, 16 cc, C] -> rows/cols
        o = res.results[c]["out"].reshape(2, 16, 16, 16, C)
        og[32 * c : 32 * c + 32] = o.transpose(0, 2, 1, 3, 4).reshape(32, W, C)
    return out


# revision 21
# speedup vs baseline: 1.4876x; 1.0307x over previous
"""OCAB (overlapping cross-attention block) Trainium2 Bass kernel, v2.

Full inputs in, full outputs out; internally shards the B*nW window axis
across 8 NeuronCores (each core owns 2 window-rows = 32 image rows, with a
4-row halo for the overlapping k/v windows).

Per core:
  Phase 1+2 (fused, 21 groups of 512 tokens): stream x, LayerNorm
  (stats on vector, normalize on scalar Identity act with per-partition
  scale/bias), PE-transpose to channel-major group tiles (ones channel at
  row 180 rides the transposes), then 5 projection passes per group into
  persistent transposed slabs: q h0-3, k h0-3, v h0-3, (q45|k45) stacked,
  v h4-5. Biases ride as an extra input channel; LN gamma/beta and the
  attention scale are folded into the weights on host.

  Phase 3 (32 windows, software-pipelined): per window materialize
  contiguous k^T/v^T windows (vector copies), build token-major v tiles
  with DMA transposes, compute S^T per head with row-packed matmuls into
  a double-buffered 3-bank PSUM ring, exp on scalar (one act per key
  chunk), chunk-major attn@V accumulation (denominators ride as ones
  columns of v), denominator broadcast via constant matmuls, fast approx
  reciprocal, projection, residual add, contiguous window-major DRAM IO
  (host pre/post-permutes the window order).
"""

import os
import sys
from contextlib import ExitStack

import numpy as np
import ml_dtypes

for _p in ("/opt/trn_rl_repo", "/root/.axon_site/_ro/trn_rl_repo"):
    if os.path.isdir(_p) and _p not in sys.path:
        sys.path.append(_p)

import concourse.bass as bass
import concourse.tile as tile
from concourse import bacc, mybir
from concourse.bass_utils import run_bass_kernel_spmd

BF16 = mybir.dt.bfloat16
F32 = mybir.dt.float32
F32R = mybir.dt.float32r
bfnp = ml_dtypes.bfloat16
AF = mybir.ActivationFunctionType
ALU = mybir.AluOpType

# ---- problem constants (hardcoded per contract) ----
C = 180
NH = 6
HD = 30
WS = 16
OWS = 24
PADW = 4
H = W = 256
EPS = 1e-5
NCORES = 8

# ---- per-core slab geometry ----
RS = 40                    # slab image rows (32 + 2*4 halo)
CS = 264                   # slab image cols (256 + 2*4 zero pad)
TS_REAL = RS * CS          # 10560 real slab tokens
TCH = 84                   # token chunks of 128
T = TCH * 128              # 10752 padded slab tokens
NG = 21                    # 512-token groups
NWIN = 32                  # windows per core (2 window-rows x 16)

KOFF = [0, 128, 256, 384, 512]     # key chunk offsets
KC = [128, 128, 128, 128, 64]      # keys per chunk
# head -> column in es sbuf
EC = {0: 0, 4: 256, 1: 512, 5: 768, 2: 1024, 3: 1280}
# head -> column in S^T psum: h0/h4/h1/h5 in st_a, h2/h3 in st_b
STC = {0: 0, 4: 256, 1: 512, 5: 768, 2: 0, 3: 512}
# head -> PE row group (must equal stationary base partition)
TPR = {0: 0, 1: 32, 2: 64, 3: 96, 4: 0, 5: 32}
STW = [[0, 1, 2], [4, 5, 3]]       # S^T emission waves (distinct banks)
# head -> column block in token-major vw tiles
VC = {0: 0, 1: 32, 2: 64, 3: 96, 4: 128, 5: 160}
# attn@V jobs: (head, av row offset, av col offset)
AVJ = [(0, 0, 0), (1, 32, 0), (2, 64, 0), (3, 96, 0), (4, 0, 256), (5, 32, 256)]

LAST_RESULTS = None
_CACHED = None


def _build_program():
    nc = bacc.Bacc("TRN2", target_bir_lowering=False)

    xs_d = nc.declare_dram_parameter("xs", [T, C], F32, isOutput=False)
    xrw_d = nc.declare_dram_parameter("xrw", [8192, C], F32, isOutput=False)
    wqkv_d = nc.declare_dram_parameter("wqkv", [181, 576], BF16, isOutput=False)
    wp_d = nc.declare_dram_parameter("wp", [192, C], BF16, isOutput=False)
    id_d = nc.declare_dram_parameter("ident", [128, 128], BF16, isOutput=False)
    e128_d = nc.declare_dram_parameter("e128", [128, 128], F32R, isOutput=False)
    e64_d = nc.declare_dram_parameter("e64", [64, 64], F32R, isOutput=False)
    out_d = nc.declare_dram_parameter("out", [8192, C], F32, isOutput=True)
    dbg_d = nc.declare_dram_parameter("dbg", [128, 2048], F32, isOutput=True)
    kstage = os.environ.get("KSTAGE", "")

    with ExitStack() as ctx:
        tc = ctx.enter_context(tile.TileContext(nc))

        wp_pool = ctx.enter_context(tc.tile_pool(name="wts", bufs=1))
        WQKV0 = wp_pool.tile([128, 576], BF16, tag="wqkv0")
        WQKV1 = wp_pool.tile([53, 576], BF16, tag="wqkv1")
        WP0 = wp_pool.tile([128, C], BF16, tag="wp0")
        WP1 = wp_pool.tile([64, C], BF16, tag="wp1")
        IDT = wp_pool.tile([128, 128], BF16, tag="id")
        E128 = wp_pool.tile([128, 128], F32R, tag="e128")
        E64 = wp_pool.tile([64, 64], F32R, tag="e64")

        nc.sync.dma_start(WQKV0[:], wqkv_d[0:128, :])
        nc.sync.dma_start(WQKV1[:], wqkv_d[128:181, :])
        nc.sync.dma_start(WP0[:], wp_d[0:128, :])
        nc.sync.dma_start(WP1[:], wp_d[128:192, :])
        nc.sync.dma_start(IDT[:], id_d[:, :])
        nc.sync.dma_start(E128[:], e128_d[:, :])
        nc.sync.dma_start(E64[:], e64_d[:, :])

        # persistent transposed slabs
        slab = ctx.enter_context(tc.tile_pool(name="slab", bufs=1))
        Q03 = slab.tile([128, T], BF16, tag="q03")
        K03 = slab.tile([128, T], BF16, tag="k03")
        V03 = slab.tile([128, T], BF16, tag="v03")
        QK45 = slab.tile([128, T], BF16, tag="qk45")   # rows 0:64 q45, 64:128 k45
        V45 = slab.tile([64, T], BF16, tag="v45")
        SLABS = [Q03, K03, V03, QK45, V45]
        SLAB_ROWS = [128, 128, 128, 128, 64]
        # which engine copies each pass's psum to its slab
        COPY_ENG = ["scalar", "scalar", "scalar", "vector", "vector"]

        # ================= phase 1+2: LN + transpose + projections ==========
        with ExitStack() as pctx:
            p_x = pctx.enter_context(tc.tile_pool(name="p_x", bufs=4))
            p_sm = pctx.enter_context(tc.tile_pool(name="p_sm", bufs=4))
            p_tp = pctx.enter_context(tc.tile_pool(name="p_tp", bufs=2, space="PSUM"))
            p_qp = pctx.enter_context(tc.tile_pool(name="p_qp", bufs=3, space="PSUM"))
            p_xtg = pctx.enter_context(tc.tile_pool(name="p_xtg", bufs=3))

            for g in range(NG):
                tp = p_tp.tile([128, 512], BF16, tag="tp")
                tp2 = p_tp.tile([128, 512], BF16, tag="tp2")
                for j in range(4):
                    tch = 4 * g + j
                    xt = p_x.tile([128, C], F32, tag="x")
                    nc.sync.dma_start(xt[:], xs_d[128 * tch : 128 * (tch + 1), :])
                    stats = p_sm.tile([128, 6], F32, tag="st")
                    aggr = p_sm.tile([128, 2], F32, tag="ag")
                    nc.vector.bn_stats(stats[:], xt[:])
                    nc.vector.bn_aggr(aggr[:], stats[:])
                    vpe = p_sm.tile([128, 1], F32, tag="vpe")
                    nc.gpsimd.tensor_scalar_add(vpe[:], aggr[:, 1:2], EPS)
                    sd = p_sm.tile([128, 1], F32, tag="sd")
                    nc.scalar.activation(sd[:], vpe[:], AF.Sqrt, bias=0.0)
                    rstd = p_sm.tile([128, 1], F32, tag="rstd")
                    nc.vector.reciprocal(rstd[:], sd[:])
                    xn = p_x.tile([128, 256], BF16, tag="xn")
                    if int(os.environ.get("KLNV", "1")):
                        nc.vector.tensor_scalar(
                            xn[:, 0:C], xt[:], aggr[:, 0:1], rstd[:],
                            op0=ALU.subtract, op1=ALU.mult,
                        )
                    else:
                        nbias = p_sm.tile([128, 1], F32, tag="nb")
                        nc.vector.scalar_tensor_tensor(
                            nbias[:], aggr[:, 0:1], -1.0, rstd[:],
                            op0=ALU.mult, op1=ALU.mult,
                        )
                        nc.scalar.activation(
                            xn[:, 0:C], xt[:], AF.Identity,
                            bias=nbias[:], scale=rstd[:],
                        )
                    nc.gpsimd.memset(xn[:, C : C + 1], 1.0)
                    nc.tensor.transpose(
                        tp[:, 128 * j : 128 * (j + 1)], xn[:, 0:128], IDT[:]
                    )
                    nc.tensor.transpose(
                        tp2[:, 128 * j : 128 * (j + 1)], xn[:, 128:256], IDT[:]
                    )
                xt0g = p_xtg.tile([128, 512], BF16, tag="xt0g")
                xt1g = p_xtg.tile([128, 512], BF16, tag="xt1g")
                nc.vector.tensor_copy(xt0g[:], tp[:])
                nc.vector.tensor_copy(xt1g[0:53, :], tp2[0:53, :])

                for p in range(5):
                    c0, c1 = (128 * p, 128 * p + 128) if p < 4 else (512, 576)
                    outw = SLAB_ROWS[p]
                    qp = p_qp.tile([128, 512], F32, tag="qp")
                    nc.tensor.matmul(
                        qp[0:outw, :], WQKV0[:, c0:c1], xt0g[:],
                        start=True, stop=False,
                    )
                    nc.tensor.matmul(
                        qp[0:outw, :], WQKV1[:, c0:c1], xt1g[0:53, :],
                        start=False, stop=True,
                    )
                    dst = SLABS[p][0:outw, 512 * g : 512 * (g + 1)]
                    if COPY_ENG[p] == "scalar":
                        nc.scalar.copy(dst, qp[0:outw, :])
                    else:
                        nc.vector.tensor_copy(dst, qp[0:outw, :])

        # ================= phase 3: windowed attention =======================
        with ExitStack() as actx:
            a_sta = actx.enter_context(tc.tile_pool(name="a_sta", bufs=2, space="PSUM"))
            a_stb = actx.enter_context(tc.tile_pool(name="a_stb", bufs=1, space="PSUM"))
            a_ep = actx.enter_context(tc.tile_pool(name="a_ep", bufs=1, space="PSUM"))
            a_avp = actx.enter_context(tc.tile_pool(name="a_avp", bufs=1, space="PSUM"))
            a_kv = actx.enter_context(tc.tile_pool(name="a_kv", bufs=2))
            a_vw = actx.enter_context(tc.tile_pool(name="a_vw", bufs=2))
            a_es = actx.enter_context(tc.tile_pool(name="a_es", bufs=7))
            a_sb = actx.enter_context(tc.tile_pool(name="a_sb", bufs=2))

            q03_pat = Q03[:, 0:TS_REAL].rearrange("p (r c) -> p r c", c=CS)
            k03_pat = K03[:, 0:TS_REAL].rearrange("p (r c) -> p r c", c=CS)
            v03_pat = V03[:, 0:TS_REAL].rearrange("p (r c) -> p r c", c=CS)
            qk45_pat = QK45[:, 0:TS_REAL].rearrange("p (r c) -> p r c", c=CS)
            v45_pat = V45[:, 0:TS_REAL].rearrange("p (r c) -> p r c", c=CS)

            # per-window state carried across pipeline stages
            wstate = {}

            for w in range(NWIN + 2):
                cur = w < NWIN
                prev = 1 <= w <= NWIN
                if cur:
                    wrl, wc = w // 16, w % 16
                    r0, c0 = WS * wrl, WS * wc
                if prev:
                    st_prev = wstate[w - 1]

                # ---- 1. denominator broadcast for window w-1 ----
                if prev:
                    P = st_prev
                    ex = a_ep.tile([128, 512], F32, tag="ep", name=f"ex{w}")
                    nc.tensor.matmul(
                        ex[:, 0:256], E128[:], P["rsb"][:, 0:256],
                        start=True, stop=True,
                    )
                    nc.tensor.matmul(
                        ex[0:64, 256:512], E64[:], P["rsb"][0:64, 256:512],
                        start=True, stop=True,
                    )

                # ---- 2. k^T / v^T window materialization (vector) ----
                if cur:
                    kw0 = a_kv.tile([128, 576], BF16, tag="kw0", name=f"kw0_{w}")
                    kw1 = a_kv.tile([64, 576], BF16, tag="kw1", name=f"kw1_{w}")
                    vwT0 = a_kv.tile([128, 640], BF16, tag="vwT0", name=f"vwT0_{w}")
                    vwT1 = a_kv.tile([64, 640], BF16, tag="vwT1", name=f"vwT1_{w}")
                    nc.vector.tensor_copy(
                        kw0[:].rearrange("p (r c) -> p r c", c=OWS),
                        k03_pat[:, r0 : r0 + OWS, c0 : c0 + OWS],
                    )
                    nc.vector.tensor_copy(
                        kw1[:].rearrange("p (r c) -> p r c", c=OWS),
                        qk45_pat[64:128, r0 : r0 + OWS, c0 : c0 + OWS],
                    )
                    nc.vector.tensor_copy(
                        vwT0[:, 0:576].rearrange("p (r c) -> p r c", c=OWS),
                        v03_pat[:, r0 : r0 + OWS, c0 : c0 + OWS],
                    )
                    nc.gpsimd.memset(vwT0[:, 576:640], 0.0)
                    nc.vector.tensor_copy(
                        vwT1[:, 0:576].rearrange("p (r c) -> p r c", c=OWS),
                        v45_pat[:, r0 : r0 + OWS, c0 : c0 + OWS],
                    )
                    nc.gpsimd.memset(vwT1[:, 576:640], 0.0)

                # ---- 3. xres load; token-major v tiles via DMA transpose ----
                if prev:
                    xres = a_sb.tile([128, 360], F32, tag="xres", name=f"xres{w}")
                    nc.sync.dma_start(
                        xres[:].rearrange("p (j d) -> p j d", d=C),
                        xrw_d[256 * (w - 1) : 256 * w, :].rearrange(
                            "(j p) d -> p j d", p=128
                        ),
                    )
                if cur:
                    vw = [
                        a_vw.tile([128, 192], BF16, tag=f"vw{i}", name=f"vw{w}_{i}")
                        for i in range(5)
                    ]
                    for i, off in enumerate(KOFF):
                        nc.sync.dma_start_transpose(
                            vw[i][:, 0:128], vwT0[:, off : off + 128]
                        )
                        eng = nc.sync if i < 3 else nc.scalar
                        eng.dma_start_transpose(
                            vw[i][:, 128:192], vwT1[:, off : off + 128]
                        )

                # ---- 4. normalize window w-1 (vector) ----
                if prev:
                    P = st_prev
                    exsb = a_sb.tile([128, 512], F32, tag="exsb", name=f"exsb{w}")
                    if int(os.environ.get("KRECIP", "1")):
                        nc.vector.reciprocal_approx_fast(exsb[:, 0:256], ex[:, 0:256])
                        nc.vector.reciprocal_approx_fast(
                            exsb[0:64, 256:512], ex[0:64, 256:512]
                        )
                    else:
                        nc.vector.reciprocal(exsb[:, 0:256], ex[:, 0:256])
                        nc.vector.reciprocal(exsb[0:64, 256:512], ex[0:64, 256:512])
                    att = a_sb.tile([128, 512], BF16, tag="att", name=f"att{w}")
                    nc.vector.tensor_tensor(
                        att[:, 0:256], P["av"][:, 0:256], exsb[:, 0:256],
                        op=ALU.mult,
                    )
                    nc.vector.tensor_tensor(
                        att[0:64, 256:512], P["av"][0:64, 256:512],
                        exsb[0:64, 256:512], op=ALU.mult,
                    )

                # ---- 5. S^T + exp + attn@V for window w ----
                if cur:
                    av = a_avp.tile([128, 512], F32, tag="av", name=f"av{w}")
                    es_list = []

                    def st_chunk(ch):
                        off, kc = KOFF[ch], KC[ch]
                        sta = a_sta.tile(
                            [128, 1024], F32, tag="sta", name=f"sta{w}_{ch}"
                        )
                        stb = a_stb.tile(
                            [128, 1024], F32, tag="stb", name=f"stb{w}_{ch}"
                        )
                        for wave in STW:
                            for h in wave:
                                if h < 4:
                                    ktile, qtile = kw0, q03_pat
                                    kr, qr = 32 * h, 32 * h
                                else:
                                    ktile, qtile = kw1, qk45_pat
                                    kr, qr = 32 * (h - 4), 32 * (h - 4)
                                st = sta if h in (0, 1, 4, 5) else stb
                                nc.tensor.matmul(
                                    st[0:kc, STC[h] : STC[h] + 256],
                                    ktile[kr : kr + 32, off : off + kc],
                                    qtile[
                                        qr : qr + 32,
                                        PADW + r0 : PADW + r0 + WS,
                                        PADW + c0 : PADW + c0 + WS,
                                    ],
                                    start=True, stop=True,
                                    tile_position=(TPR[h], 0),
                                )
                        return sta, stb

                    def exp_chunk(ch, sta, stb):
                        kc = KC[ch]
                        es = a_es.tile(
                            [128, 1536], BF16, tag="es", name=f"es{w}_{ch}"
                        )
                        nc.scalar.activation(
                            es[0:kc, 0:1024], sta[0:kc, :], AF.Exp
                        )
                        nc.scalar.activation(
                            es[0:kc, 1024:1536].rearrange(
                                "p (a b) -> p a b", b=256
                            ),
                            stb[0:kc, :].rearrange("p (a b) -> p a b", b=512)[
                                :, :, 0:256
                            ],
                            AF.Exp,
                        )
                        es_list.append(es)

                    def av_one(h, ro, co, ch):
                        # jobs with disjoint psum partitions may hold
                        # concurrently-open accumulation groups; jobs sharing
                        # partitions+bank (h0/h4, h1/h5) must not interleave
                        kc = KC[ch]
                        nc.tensor.matmul(
                            av[ro : ro + 32, co : co + 256],
                            vw[ch][0:kc, VC[h] : VC[h] + 32],
                            es_list[ch][0:kc, EC[h] : EC[h] + 256],
                            start=(ch == 0), stop=(ch == 4),
                            tile_position=(0, ro),
                            skip_group_check=True,
                        )

                    def av_chunk(ch):
                        for h, ro, co in AVJ[:4]:
                            av_one(h, ro, co, ch)

                    sts = [st_chunk(0)]
                    exp_chunk(0, *sts[0])
                    sts.append(st_chunk(1))
                    exp_chunk(1, *sts[1])
                    av_chunk(0)
                    sts.append(st_chunk(2))
                    exp_chunk(2, *sts[2])
                    av_chunk(1)
                    sts.append(st_chunk(3))
                    exp_chunk(3, *sts[3])
                    av_chunk(2)
                    sts.append(st_chunk(4))
                    exp_chunk(4, *sts[4])
                    av_chunk(3)
                    av_chunk(4)
                    for ch in range(5):
                        for h, ro, co in AVJ[4:]:
                            av_one(h, ro, co, ch)

                # ---- 6. projection + residual + store for window w-1 ----
                if prev:
                    P = st_prev
                    pp = a_ep.tile([128, 360], F32, tag="ep", name=f"pp{w}")
                    for qc in range(2):
                        nc.tensor.matmul(
                            pp[:, 180 * qc : 180 * qc + 180],
                            att[:, 128 * qc : 128 * (qc + 1)],
                            WP0[:],
                            start=True, stop=False,
                        )
                        nc.tensor.matmul(
                            pp[:, 180 * qc : 180 * qc + 180],
                            att[0:64, 256 + 128 * qc : 256 + 128 * (qc + 1)],
                            WP1[:],
                            start=False, stop=True,
                        )
                    ot = a_sb.tile([128, 360], F32, tag="ot", name=f"ot{w}", bufs=3)
                    nc.vector.tensor_tensor(ot[:], pp[:], xres[:], op=ALU.add)
                    st_prev["ot"] = ot

                # ---- 6b. store window w-2 (data long ready: no queue block)
                if 2 <= w <= NWIN + 1:
                    nc.sync.dma_start(
                        out_d[256 * (w - 2) : 256 * (w - 1), :].rearrange(
                            "(j p) d -> p j d", p=128
                        ),
                        wstate[w - 2]["ot"][:].rearrange("p (j d) -> p j d", d=C),
                    )

                # ---- debug dumps for window 0 ----
                if cur and w == 0 and kstage:
                    dbg = a_sb.tile([128, 2048], F32, tag="dbg", bufs=1)
                    if kstage == "q":
                        nc.vector.tensor_copy(dbg[:, 0:2048], Q03[:, 0:2048])
                    elif kstage == "kw":
                        nc.vector.tensor_copy(dbg[:, 0:576], kw0[:])
                        nc.vector.tensor_copy(dbg[0:64, 576:1152], kw1[:])
                    elif kstage == "vw0":
                        nc.vector.tensor_copy(dbg[:, 0:192], vw[0][:])
                        nc.vector.tensor_copy(dbg[:, 192:384], vw[4][:])
                    elif kstage == "es0":
                        nc.vector.tensor_copy(dbg[:, 0:1536], es_list[0][:])
                    elif kstage == "es4":
                        nc.vector.tensor_copy(dbg[0:64, 0:1536], es_list[4][0:64, :])
                    elif kstage == "av":
                        nc.vector.tensor_copy(dbg[:, 0:512], av[:])
                    nc.sync.dma_start(dbg_d[:, :], dbg[:])

                # ---- 7. rowsum snapshot for window w ----
                if cur:
                    rsb = a_sb.tile([128, 512], F32R, tag="rsb", name=f"rsb{w}")
                    nc.vector.tensor_copy(rsb[:], av[:])
                    wstate[w] = {"av": av, "rsb": rsb}
                    wstate.pop(w - 3, None)

    nc.compile()
    return nc


def _prep_host(inputs):
    x = np.ascontiguousarray(inputs["x"], dtype=np.float32)[0]  # [65536, 180]
    norm_w = np.asarray(inputs["norm_w"], np.float32)
    norm_b = np.asarray(inputs["norm_b"], np.float32)
    q_w = np.asarray(inputs["q_w"], np.float32)
    q_b = np.asarray(inputs["q_b"], np.float32)
    kv_w = np.asarray(inputs["kv_w"], np.float32)
    kv_b = np.asarray(inputs["kv_b"], np.float32)
    proj_w = np.asarray(inputs["proj_w"], np.float32)
    proj_b = np.asarray(inputs["proj_b"], np.float32)

    scale = HD ** -0.5
    Wq = norm_w[:, None] * q_w * scale
    bq = (norm_b @ q_w + q_b) * scale
    Wk = norm_w[:, None] * kv_w[:, :C]
    bk = norm_b @ kv_w[:, :C] + kv_b[:C]
    Wv = norm_w[:, None] * kv_w[:, C:]
    bv = norm_b @ kv_w[:, C:] + kv_b[C:]

    # wqkv [181, 576]: q03 | k03 | v03 | (q45|k45) | v45, 32-col head blocks;
    # v blocks carry 1.0 at row 180 in cols 30/31 (denominator ride-along)
    wqkv = np.zeros((181, 576), np.float32)

    def put(colbase, h, Wm, bm, ones):
        col = colbase + 32 * (h % 4)
        wqkv[:C, col : col + HD] = Wm[:, HD * h : HD * (h + 1)]
        wqkv[C, col : col + HD] = bm[HD * h : HD * (h + 1)]
        if ones:
            wqkv[C, col + 30] = 1.0
            wqkv[C, col + 31] = 1.0

    for h in range(4):
        put(0, h, Wq, bq, False)
        put(128, h, Wk, bk, False)
        put(256, h, Wv, bv, True)
    for h in (4, 5):
        put(384, h, Wq, bq, False)
        put(448, h, Wk, bk, False)
        put(512, h, Wv, bv, True)

    # wp [192, 180]: rows 0:128 = proj rows h0-3 (32-blocks), 128:192 h4-5
    wp = np.zeros((192, C), np.float32)
    for h in range(NH):
        row = 32 * h if h < 4 else 128 + 32 * (h - 4)
        wp[row : row + HD, :] = proj_w[HD * h : HD * (h + 1), :]

    ident = np.eye(128, dtype=bfnp)
    e128 = np.zeros((128, 128), np.float32)
    for j in range(4):
        e128[32 * j + 30, 32 * j : 32 * j + 32] = 1.0
    e64 = np.zeros((64, 64), np.float32)
    e64[30, 0:32] = 1.0
    e64[62, 32:64] = 1.0

    # per-core slabs
    xg = x.reshape(H, W, C)
    xpad = np.zeros((H + 2 * PADW, CS, C), np.float32)
    xpad[PADW : PADW + H, PADW : PADW + W, :] = xg
    xres_full = (x + proj_b).reshape(H, W, C)

    in_maps = []
    for c in range(NCORES):
        slab_x = np.zeros((T, C), np.float32)
        slab_x[:TS_REAL] = xpad[32 * c : 32 * c + RS].reshape(TS_REAL, C)
        # window-major residual: [2 wrl, 16 r, 16 wc, 16 cc, C] ->
        # (wrl, wc, r, cc)
        xr = xres_full[32 * c : 32 * c + 32].reshape(2, 16, 16, 16, C)
        xr = np.ascontiguousarray(xr.transpose(0, 2, 1, 3, 4)).reshape(8192, C)
        in_maps.append(
            {
                "xs": slab_x,
                "xrw": xr,
                "wqkv": wqkv.astype(bfnp),
                "wp": wp.astype(bfnp),
                "ident": ident,
                "e128": e128,
                "e64": e64,
            }
        )
    return in_maps


def kernel(**inputs):
    global _CACHED, LAST_RESULTS
    if _CACHED is None:
        _CACHED = _build_program()
    nc = _CACHED
    in_maps = _prep_host(inputs)
    res = run_bass_kernel_spmd(
        nc,
        in_maps,
        list(range(NCORES)),
        trace=bool(int(os.environ.get("KTRACE", "0"))),
    )
    LAST_RESULTS = res
    out = np.empty((1, H * W, C), np.float32)
    og = out[0].reshape(H, W, C)
    for c in range(NCORES):
        # un-permute window-major [2, 16 wc, 16 r, 16 cc, C] -> rows/cols
        o = res.results[c]["out"].reshape(2, 16, 16, 16, C)
        og[32 * c : 32 * c + 32] = o.transpose(0, 2, 1, 3, 4).reshape(32, W, C)
    return out


# revision 24
# speedup vs baseline: 1.9080x; 1.2826x over previous
"""OCAB (overlapping cross-attention block) Trainium2 Bass kernel, v2.

Full inputs in, full outputs out; internally shards the B*nW window axis
across 8 NeuronCores (each core owns 2 window-rows = 32 image rows, with a
4-row halo for the overlapping k/v windows).

Per core:
  Phase 1+2 (fused, 21 groups of 512 tokens): stream x, LayerNorm
  (stats on vector, normalize on scalar Identity act with per-partition
  scale/bias), PE-transpose to channel-major group tiles (ones channel at
  row 180 rides the transposes), then 5 projection passes per group into
  persistent transposed slabs: q h0-3, k h0-3, v h0-3, (q45|k45) stacked,
  v h4-5. Biases ride as an extra input channel; LN gamma/beta and the
  attention scale are folded into the weights on host.

  Phase 3 (32 windows, software-pipelined): per window materialize
  contiguous k^T/v^T windows (vector copies), build token-major v tiles
  with DMA transposes, compute S^T per head with row-packed matmuls into
  a double-buffered 3-bank PSUM ring, exp on scalar (one act per key
  chunk), chunk-major attn@V accumulation (denominators ride as ones
  columns of v), denominator broadcast via constant matmuls, fast approx
  reciprocal, projection, residual add, contiguous window-major DRAM IO
  (host pre/post-permutes the window order).
"""

import os
import sys
from contextlib import ExitStack

import numpy as np
import ml_dtypes

for _p in ("/opt/trn_rl_repo", "/root/.axon_site/_ro/trn_rl_repo"):
    if os.path.isdir(_p) and _p not in sys.path:
        sys.path.append(_p)

import concourse.bass as bass
import concourse.tile as tile
from concourse import bacc, mybir
from concourse.bass_utils import run_bass_kernel_spmd

BF16 = mybir.dt.bfloat16
F32 = mybir.dt.float32
F32R = mybir.dt.float32r
bfnp = ml_dtypes.bfloat16
AF = mybir.ActivationFunctionType
ALU = mybir.AluOpType

# ---- problem constants (hardcoded per contract) ----
C = 180
NH = 6
HD = 30
WS = 16
OWS = 24
PADW = 4
H = W = 256
EPS = 1e-5
NCORES = 8

# ---- per-core slab geometry ----
RS = 40                    # slab image rows (32 + 2*4 halo)
CS = 264                   # slab image cols (256 + 2*4 zero pad)
TS_REAL = RS * CS          # 10560 real slab tokens
TCH = 84                   # token chunks of 128
T = TCH * 128              # 10752 padded slab tokens
NG = 21                    # 512-token groups
NWIN = 32                  # windows per core (2 window-rows x 16)

KOFF = [0, 128, 256, 384, 512]     # key chunk offsets
KC = [128, 128, 128, 128, 64]      # keys per chunk
# head -> column in es sbuf
EC = {0: 0, 4: 256, 1: 512, 5: 768, 2: 1024, 3: 1280}
# head -> column in S^T psum: h0/h4/h1/h5 in st_a, h2/h3 in st_b
STC = {0: 0, 4: 256, 1: 512, 5: 768, 2: 0, 3: 512}
# head -> PE row group (must equal stationary base partition)
TPR = {0: 0, 1: 32, 2: 64, 3: 96, 4: 0, 5: 32}
STW = [[0, 1, 2], [4, 5, 3]]       # S^T emission waves (distinct banks)
# head -> column block in token-major vw tiles
VC = {0: 0, 1: 32, 2: 64, 3: 96, 4: 128, 5: 160}
# attn@V jobs: (head, av row offset, av col offset)
AVJ = [(0, 0, 0), (1, 32, 0), (2, 64, 0), (3, 96, 0), (4, 0, 256), (5, 32, 256)]

LAST_RESULTS = None
_CACHED = None


def _build_program():
    nc = bacc.Bacc("TRN2", target_bir_lowering=False)

    xs_d = nc.declare_dram_parameter("xs", [T, C], F32, isOutput=False)
    xrw_d = nc.declare_dram_parameter("xrw", [8192, C], F32, isOutput=False)
    wqkv_d = nc.declare_dram_parameter("wqkv", [181, 576], BF16, isOutput=False)
    wp_d = nc.declare_dram_parameter("wp", [192, C], BF16, isOutput=False)
    id_d = nc.declare_dram_parameter("ident", [128, 128], BF16, isOutput=False)
    e128_d = nc.declare_dram_parameter("e128", [128, 128], F32R, isOutput=False)
    e64_d = nc.declare_dram_parameter("e64", [64, 64], F32R, isOutput=False)
    out_d = nc.declare_dram_parameter("out", [8192, C], F32, isOutput=True)
    dbg_d = nc.declare_dram_parameter("dbg", [128, 2048], F32, isOutput=True)
    kstage = os.environ.get("KSTAGE", "")

    with ExitStack() as ctx:
        tc = ctx.enter_context(tile.TileContext(nc))

        wp_pool = ctx.enter_context(tc.tile_pool(name="wts", bufs=1))
        WQKV0 = wp_pool.tile([128, 576], BF16, tag="wqkv0")
        WQKV1 = wp_pool.tile([53, 576], BF16, tag="wqkv1")
        WP0 = wp_pool.tile([128, C], BF16, tag="wp0")
        WP1 = wp_pool.tile([64, C], BF16, tag="wp1")
        IDT = wp_pool.tile([128, 128], BF16, tag="id")
        E128 = wp_pool.tile([128, 128], F32R, tag="e128")
        E64 = wp_pool.tile([64, 64], F32R, tag="e64")

        nc.sync.dma_start(WQKV0[:], wqkv_d[0:128, :])
        nc.sync.dma_start(WQKV1[:], wqkv_d[128:181, :])
        nc.sync.dma_start(WP0[:], wp_d[0:128, :])
        nc.sync.dma_start(WP1[:], wp_d[128:192, :])
        nc.sync.dma_start(IDT[:], id_d[:, :])
        nc.sync.dma_start(E128[:], e128_d[:, :])
        nc.sync.dma_start(E64[:], e64_d[:, :])

        # persistent transposed slabs
        slab = ctx.enter_context(tc.tile_pool(name="slab", bufs=1))
        Q03 = slab.tile([128, T], BF16, tag="q03")
        K03 = slab.tile([128, T], BF16, tag="k03")
        V03 = slab.tile([128, T], BF16, tag="v03")
        QK45 = slab.tile([128, T], BF16, tag="qk45")   # rows 0:64 q45, 64:128 k45
        V45 = slab.tile([64, T], BF16, tag="v45")
        SLABS = [Q03, K03, V03, QK45, V45]
        SLAB_ROWS = [128, 128, 128, 128, 64]
        # which engine copies each pass's psum to its slab
        COPY_ENG = ["scalar", "scalar", "scalar", "vector", "vector"]

        # ================= phase 1+2: LN + transpose + projections ==========
        with ExitStack() as pctx:
            p_x = pctx.enter_context(tc.tile_pool(name="p_x", bufs=4))
            p_sm = pctx.enter_context(tc.tile_pool(name="p_sm", bufs=4))
            p_tp = pctx.enter_context(tc.tile_pool(name="p_tp", bufs=2, space="PSUM"))
            p_qp = pctx.enter_context(tc.tile_pool(name="p_qp", bufs=3, space="PSUM"))
            p_xtg = pctx.enter_context(tc.tile_pool(name="p_xtg", bufs=3))

            for g in range(NG):
                tp = p_tp.tile([128, 512], BF16, tag="tp")
                tp2 = p_tp.tile([128, 512], BF16, tag="tp2")
                for j in range(4):
                    tch = 4 * g + j
                    xt = p_x.tile([128, C], F32, tag="x")
                    nc.sync.dma_start(xt[:], xs_d[128 * tch : 128 * (tch + 1), :])
                    stats = p_sm.tile([128, 6], F32, tag="st")
                    aggr = p_sm.tile([128, 2], F32, tag="ag")
                    nc.vector.bn_stats(stats[:], xt[:])
                    nc.vector.bn_aggr(aggr[:], stats[:])
                    vpe = p_sm.tile([128, 1], F32, tag="vpe")
                    nc.gpsimd.tensor_scalar_add(vpe[:], aggr[:, 1:2], EPS)
                    sd = p_sm.tile([128, 1], F32, tag="sd")
                    nc.scalar.activation(sd[:], vpe[:], AF.Sqrt, bias=0.0)
                    rstd = p_sm.tile([128, 1], F32, tag="rstd")
                    nc.vector.reciprocal(rstd[:], sd[:])
                    xn = p_x.tile([128, 256], BF16, tag="xn")
                    if int(os.environ.get("KLNV", "1")):
                        nc.vector.tensor_scalar(
                            xn[:, 0:C], xt[:], aggr[:, 0:1], rstd[:],
                            op0=ALU.subtract, op1=ALU.mult,
                        )
                    else:
                        nbias = p_sm.tile([128, 1], F32, tag="nb")
                        nc.vector.scalar_tensor_tensor(
                            nbias[:], aggr[:, 0:1], -1.0, rstd[:],
                            op0=ALU.mult, op1=ALU.mult,
                        )
                        nc.scalar.activation(
                            xn[:, 0:C], xt[:], AF.Identity,
                            bias=nbias[:], scale=rstd[:],
                        )
                    nc.gpsimd.memset(xn[:, C : C + 1], 1.0)
                    nc.tensor.transpose(
                        tp[:, 128 * j : 128 * (j + 1)], xn[:, 0:128], IDT[:]
                    )
                    nc.tensor.transpose(
                        tp2[:, 128 * j : 128 * (j + 1)], xn[:, 128:256], IDT[:]
                    )
                xt0g = p_xtg.tile([128, 512], BF16, tag="xt0g")
                xt1g = p_xtg.tile([128, 512], BF16, tag="xt1g")
                nc.vector.tensor_copy(xt0g[:], tp[:])
                nc.vector.tensor_copy(xt1g[0:53, :], tp2[0:53, :])

                for p in range(5):
                    c0, c1 = (128 * p, 128 * p + 128) if p < 4 else (512, 576)
                    outw = SLAB_ROWS[p]
                    qp = p_qp.tile([128, 512], F32, tag="qp")
                    nc.tensor.matmul(
                        qp[0:outw, :], WQKV0[:, c0:c1], xt0g[:],
                        start=True, stop=False,
                    )
                    nc.tensor.matmul(
                        qp[0:outw, :], WQKV1[:, c0:c1], xt1g[0:53, :],
                        start=False, stop=True,
                    )
                    dst = SLABS[p][0:outw, 512 * g : 512 * (g + 1)]
                    if COPY_ENG[p] == "scalar":
                        nc.scalar.copy(dst, qp[0:outw, :])
                    else:
                        nc.vector.tensor_copy(dst, qp[0:outw, :])

        # ================= phase 3: windowed attention =======================
        with ExitStack() as actx:
            a_sta = actx.enter_context(tc.tile_pool(name="a_sta", bufs=2, space="PSUM"))
            a_stb = actx.enter_context(tc.tile_pool(name="a_stb", bufs=1, space="PSUM"))
            a_ep = actx.enter_context(tc.tile_pool(name="a_ep", bufs=1, space="PSUM"))
            a_avp = actx.enter_context(tc.tile_pool(name="a_avp", bufs=1, space="PSUM"))
            a_kv = actx.enter_context(tc.tile_pool(name="a_kv", bufs=2))
            a_vw = actx.enter_context(tc.tile_pool(name="a_vw", bufs=2))
            a_es = actx.enter_context(tc.tile_pool(name="a_es", bufs=7))
            a_sb = actx.enter_context(tc.tile_pool(name="a_sb", bufs=2))

            q03_pat = Q03[:, 0:TS_REAL].rearrange("p (r c) -> p r c", c=CS)
            k03_pat = K03[:, 0:TS_REAL].rearrange("p (r c) -> p r c", c=CS)
            v03_pat = V03[:, 0:TS_REAL].rearrange("p (r c) -> p r c", c=CS)
            qk45_pat = QK45[:, 0:TS_REAL].rearrange("p (r c) -> p r c", c=CS)
            v45_pat = V45[:, 0:TS_REAL].rearrange("p (r c) -> p r c", c=CS)

            # per-window state carried across pipeline stages
            wstate = {}

            # stage window 0's k^T/v^T slices ahead of the loop
            kw0n = a_kv.tile([128, 576], BF16, tag="kw0", name="kw0_p")
            kw1n = a_kv.tile([64, 576], BF16, tag="kw1", name="kw1_p")
            vwT0n = a_kv.tile([128, 576], BF16, tag="vwT0", name="vwT0_p")
            vwT1n = a_kv.tile([64, 576], BF16, tag="vwT1", name="vwT1_p")
            nc.vector.tensor_copy(
                kw0n[:].rearrange("p (r c) -> p r c", c=OWS),
                k03_pat[:, 0:OWS, 0:OWS],
            )
            nc.vector.tensor_copy(
                kw1n[:].rearrange("p (r c) -> p r c", c=OWS),
                qk45_pat[64:128, 0:OWS, 0:OWS],
            )
            nc.vector.tensor_copy(
                vwT0n[:].rearrange("p (r c) -> p r c", c=OWS),
                v03_pat[:, 0:OWS, 0:OWS],
            )
            nc.vector.tensor_copy(
                vwT1n[:].rearrange("p (r c) -> p r c", c=OWS),
                v45_pat[:, 0:OWS, 0:OWS],
            )
            kvt_next = (kw0n, kw1n, vwT0n, vwT1n)

            for w in range(NWIN + 2):
                cur = w < NWIN
                prev = 1 <= w <= NWIN
                if cur:
                    wrl, wc = w // 16, w % 16
                    r0, c0 = WS * wrl, WS * wc
                if prev:
                    st_prev = wstate[w - 1]

                # ---- 1. denominator broadcast for window w-1 ----
                if prev:
                    P = st_prev
                    ex = a_ep.tile([128, 512], F32, tag="ep", name=f"ex{w}")
                    nc.tensor.matmul(
                        ex[:, 0:256], E128[:], P["rsb"][:, 0:256],
                        start=True, stop=True,
                    )
                    nc.tensor.matmul(
                        ex[0:64, 256:512], E64[:], P["rsb"][0:64, 256:512],
                        start=True, stop=True,
                    )

                # ---- 4. normalize window w-1 (vector, early: unblocks PE) ----
                if prev:
                    P = st_prev
                    exsb = a_sb.tile([128, 512], F32, tag="exsb", name=f"exsb{w}")
                    if int(os.environ.get("KRECIP", "1")):
                        nc.vector.reciprocal_approx_fast(exsb[:, 0:256], ex[:, 0:256])
                        nc.vector.reciprocal_approx_fast(
                            exsb[0:64, 256:512], ex[0:64, 256:512]
                        )
                    else:
                        nc.vector.reciprocal(exsb[:, 0:256], ex[:, 0:256])
                        nc.vector.reciprocal(exsb[0:64, 256:512], ex[0:64, 256:512])
                    att = a_sb.tile([128, 512], BF16, tag="att", name=f"att{w}")
                    nc.vector.tensor_tensor(
                        att[:, 0:256], P["av"][:, 0:256], exsb[:, 0:256],
                        op=ALU.mult,
                    )
                    nc.vector.tensor_tensor(
                        att[0:64, 256:512], P["av"][0:64, 256:512],
                        exsb[0:64, 256:512], op=ALU.mult,
                    )

                # ---- 3. xres load; token-major v tiles via PE transpose ----
                if prev:
                    xres = a_sb.tile([128, 360], F32, tag="xres", name=f"xres{w}")
                    nc.sync.dma_start(
                        xres[:].rearrange("p (j d) -> p j d", d=C),
                        xrw_d[256 * (w - 1) : 256 * w, :].rearrange(
                            "(j p) d -> p j d", p=128
                        ),
                    )
                if cur:
                    kw0, kw1, vwT0, vwT1 = kvt_next
                    vwp = a_ep.tile([128, 960], BF16, tag="ep", name=f"vwp{w}")
                    vw = [
                        a_vw.tile([128, 192], BF16, tag=f"vw{i}", name=f"vw{w}_{i}")
                        for i in range(5)
                    ]
                    for i, off in enumerate(KOFF):
                        kc = KC[i]
                        nc.tensor.transpose(
                            vwp[0:kc, 192 * i : 192 * i + 128],
                            vwT0[:, off : off + kc],
                            IDT[:],
                        )
                        nc.tensor.transpose(
                            vwp[0:kc, 192 * i + 128 : 192 * i + 192],
                            vwT1[:, off : off + kc],
                            IDT[0:64, 0:64],
                        )
                        nc.vector.tensor_copy(
                            vw[i][0:kc, :], vwp[0:kc, 192 * i : 192 * i + 192]
                        )

                # ---- 3b. stage k^T/v^T window slices for window w+1 ----
                if w + 1 < NWIN:
                    nwrl, nwc = (w + 1) // 16, (w + 1) % 16
                    nr0, nc0 = WS * nwrl, WS * nwc
                    kw0n = a_kv.tile([128, 576], BF16, tag="kw0", name=f"kw0_{w+1}")
                    kw1n = a_kv.tile([64, 576], BF16, tag="kw1", name=f"kw1_{w+1}")
                    vwT0n = a_kv.tile([128, 576], BF16, tag="vwT0", name=f"vwT0_{w+1}")
                    vwT1n = a_kv.tile([64, 576], BF16, tag="vwT1", name=f"vwT1_{w+1}")
                    nc.vector.tensor_copy(
                        kw0n[:].rearrange("p (r c) -> p r c", c=OWS),
                        k03_pat[:, nr0 : nr0 + OWS, nc0 : nc0 + OWS],
                    )
                    nc.vector.tensor_copy(
                        kw1n[:].rearrange("p (r c) -> p r c", c=OWS),
                        qk45_pat[64:128, nr0 : nr0 + OWS, nc0 : nc0 + OWS],
                    )
                    nc.vector.tensor_copy(
                        vwT0n[:].rearrange("p (r c) -> p r c", c=OWS),
                        v03_pat[:, nr0 : nr0 + OWS, nc0 : nc0 + OWS],
                    )
                    nc.vector.tensor_copy(
                        vwT1n[:].rearrange("p (r c) -> p r c", c=OWS),
                        v45_pat[:, nr0 : nr0 + OWS, nc0 : nc0 + OWS],
                    )
                    kvt_next = (kw0n, kw1n, vwT0n, vwT1n)

                # ---- 5. S^T + exp + attn@V for window w ----
                if cur:
                    av = a_avp.tile([128, 512], F32, tag="av", name=f"av{w}")
                    es_list = []

                    def st_chunk(ch):
                        off, kc = KOFF[ch], KC[ch]
                        sta = a_sta.tile(
                            [128, 1024], F32, tag="sta", name=f"sta{w}_{ch}"
                        )
                        stb = a_stb.tile(
                            [128, 1024], F32, tag="stb", name=f"stb{w}_{ch}"
                        )
                        for wave in STW:
                            for h in wave:
                                if h < 4:
                                    ktile, qtile = kw0, q03_pat
                                    kr, qr = 32 * h, 32 * h
                                else:
                                    ktile, qtile = kw1, qk45_pat
                                    kr, qr = 32 * (h - 4), 32 * (h - 4)
                                st = sta if h in (0, 1, 4, 5) else stb
                                nc.tensor.matmul(
                                    st[0:kc, STC[h] : STC[h] + 256],
                                    ktile[kr : kr + 32, off : off + kc],
                                    qtile[
                                        qr : qr + 32,
                                        PADW + r0 : PADW + r0 + WS,
                                        PADW + c0 : PADW + c0 + WS,
                                    ],
                                    start=True, stop=True,
                                    tile_position=(TPR[h], 0),
                                )
                        return sta, stb

                    def exp_chunk(ch, sta, stb):
                        kc = KC[ch]
                        es = a_es.tile(
                            [128, 1536], BF16, tag="es", name=f"es{w}_{ch}"
                        )
                        nc.scalar.activation(
                            es[0:kc, 0:1024], sta[0:kc, :], AF.Exp
                        )
                        nc.scalar.activation(
                            es[0:kc, 1024:1536].rearrange(
                                "p (a b) -> p a b", b=256
                            ),
                            stb[0:kc, :].rearrange("p (a b) -> p a b", b=512)[
                                :, :, 0:256
                            ],
                            AF.Exp,
                        )
                        es_list.append(es)

                    def av_one(h, ro, co, ch):
                        # jobs with disjoint psum partitions may hold
                        # concurrently-open accumulation groups; jobs sharing
                        # partitions+bank (h0/h4, h1/h5) must not interleave
                        kc = KC[ch]
                        nc.tensor.matmul(
                            av[ro : ro + 32, co : co + 256],
                            vw[ch][0:kc, VC[h] : VC[h] + 32],
                            es_list[ch][0:kc, EC[h] : EC[h] + 256],
                            start=(ch == 0), stop=(ch == 4),
                            tile_position=(0, ro),
                            skip_group_check=True,
                        )

                    def av_chunk(ch):
                        for h, ro, co in AVJ[:4]:
                            av_one(h, ro, co, ch)

                    sts = [st_chunk(0)]
                    exp_chunk(0, *sts[0])
                    sts.append(st_chunk(1))
                    exp_chunk(1, *sts[1])
                    av_chunk(0)
                    sts.append(st_chunk(2))
                    exp_chunk(2, *sts[2])
                    av_chunk(1)
                    sts.append(st_chunk(3))
                    exp_chunk(3, *sts[3])
                    av_chunk(2)
                    sts.append(st_chunk(4))
                    exp_chunk(4, *sts[4])
                    av_chunk(3)
                    av_chunk(4)
                    for ch in range(5):
                        for h, ro, co in AVJ[4:]:
                            av_one(h, ro, co, ch)

                # ---- 6. projection + residual + store for window w-1 ----
                if prev:
                    P = st_prev
                    pp = a_ep.tile([128, 360], F32, tag="ep", name=f"pp{w}")
                    for qc in range(2):
                        nc.tensor.matmul(
                            pp[:, 180 * qc : 180 * qc + 180],
                            att[:, 128 * qc : 128 * (qc + 1)],
                            WP0[:],
                            start=True, stop=False,
                        )
                        nc.tensor.matmul(
                            pp[:, 180 * qc : 180 * qc + 180],
                            att[0:64, 256 + 128 * qc : 256 + 128 * (qc + 1)],
                            WP1[:],
                            start=False, stop=True,
                        )
                    ot = a_sb.tile([128, 360], F32, tag="ot", name=f"ot{w}", bufs=3)
                    nc.vector.tensor_tensor(ot[:], pp[:], xres[:], op=ALU.add)
                    st_prev["ot"] = ot

                # ---- 6b. store window w-2 (data long ready: no queue block)
                if 2 <= w <= NWIN + 1:
                    nc.sync.dma_start(
                        out_d[256 * (w - 2) : 256 * (w - 1), :].rearrange(
                            "(j p) d -> p j d", p=128
                        ),
                        wstate[w - 2]["ot"][:].rearrange("p (j d) -> p j d", d=C),
                    )

                # ---- debug dumps for window 0 ----
                if cur and w == 0 and kstage:
                    dbg = a_sb.tile([128, 2048], F32, tag="dbg", bufs=1)
                    if kstage == "q":
                        nc.vector.tensor_copy(dbg[:, 0:2048], Q03[:, 0:2048])
                    elif kstage == "kw":
                        nc.vector.tensor_copy(dbg[:, 0:576], kw0[:])
                        nc.vector.tensor_copy(dbg[0:64, 576:1152], kw1[:])
                    elif kstage == "vw0":
                        nc.vector.tensor_copy(dbg[:, 0:192], vw[0][:])
                        nc.vector.tensor_copy(dbg[:, 192:384], vw[4][:])
                    elif kstage == "es0":
                        nc.vector.tensor_copy(dbg[:, 0:1536], es_list[0][:])
                    elif kstage == "es4":
                        nc.vector.tensor_copy(dbg[0:64, 0:1536], es_list[4][0:64, :])
                    elif kstage == "av":
                        nc.vector.tensor_copy(dbg[:, 0:512], av[:])
                    nc.sync.dma_start(dbg_d[:, :], dbg[:])

                # ---- 7. rowsum snapshot for window w ----
                if cur:
                    rsb = a_sb.tile([128, 512], F32R, tag="rsb", name=f"rsb{w}")
                    nc.vector.tensor_copy(rsb[:], av[:])
                    wstate[w] = {"av": av, "rsb": rsb}
                    wstate.pop(w - 3, None)

    nc.compile()
    return nc


def _prep_host(inputs):
    x = np.ascontiguousarray(inputs["x"], dtype=np.float32)[0]  # [65536, 180]
    norm_w = np.asarray(inputs["norm_w"], np.float32)
    norm_b = np.asarray(inputs["norm_b"], np.float32)
    q_w = np.asarray(inputs["q_w"], np.float32)
    q_b = np.asarray(inputs["q_b"], np.float32)
    kv_w = np.asarray(inputs["kv_w"], np.float32)
    kv_b = np.asarray(inputs["kv_b"], np.float32)
    proj_w = np.asarray(inputs["proj_w"], np.float32)
    proj_b = np.asarray(inputs["proj_b"], np.float32)

    scale = HD ** -0.5
    Wq = norm_w[:, None] * q_w * scale
    bq = (norm_b @ q_w + q_b) * scale
    Wk = norm_w[:, None] * kv_w[:, :C]
    bk = norm_b @ kv_w[:, :C] + kv_b[:C]
    Wv = norm_w[:, None] * kv_w[:, C:]
    bv = norm_b @ kv_w[:, C:] + kv_b[C:]

    # wqkv [181, 576]: q03 | k03 | v03 | (q45|k45) | v45, 32-col head blocks;
    # v blocks carry 1.0 at row 180 in cols 30/31 (denominator ride-along)
    wqkv = np.zeros((181, 576), np.float32)

    def put(colbase, h, Wm, bm, ones):
        col = colbase + 32 * (h % 4)
        wqkv[:C, col : col + HD] = Wm[:, HD * h : HD * (h + 1)]
        wqkv[C, col : col + HD] = bm[HD * h : HD * (h + 1)]
        if ones:
            wqkv[C, col + 30] = 1.0
            wqkv[C, col + 31] = 1.0

    for h in range(4):
        put(0, h, Wq, bq, False)
        put(128, h, Wk, bk, False)
        put(256, h, Wv, bv, True)
    for h in (4, 5):
        put(384, h, Wq, bq, False)
        put(448, h, Wk, bk, False)
        put(512, h, Wv, bv, True)

    # wp [192, 180]: rows 0:128 = proj rows h0-3 (32-blocks), 128:192 h4-5
    wp = np.zeros((192, C), np.float32)
    for h in range(NH):
        row = 32 * h if h < 4 else 128 + 32 * (h - 4)
        wp[row : row + HD, :] = proj_w[HD * h : HD * (h + 1), :]

    ident = np.eye(128, dtype=bfnp)
    e128 = np.zeros((128, 128), np.float32)
    for j in range(4):
        e128[32 * j + 30, 32 * j : 32 * j + 32] = 1.0
    e64 = np.zeros((64, 64), np.float32)
    e64[30, 0:32] = 1.0
    e64[62, 32:64] = 1.0

    # per-core slabs
    xg = x.reshape(H, W, C)
    xpad = np.zeros((H + 2 * PADW, CS, C), np.float32)
    xpad[PADW : PADW + H, PADW : PADW + W, :] = xg
    xres_full = (x + proj_b).reshape(H, W, C)

    in_maps = []
    for c in range(NCORES):
        slab_x = np.zeros((T, C), np.float32)
        slab_x[:TS_REAL] = xpad[32 * c : 32 * c + RS].reshape(TS_REAL, C)
        # window-major residual: [2 wrl, 16 r, 16 wc, 16 cc, C] ->
        # (wrl, wc, r, cc)
        xr = xres_full[32 * c : 32 * c + 32].reshape(2, 16, 16, 16, C)
        xr = np.ascontiguousarray(xr.transpose(0, 2, 1, 3, 4)).reshape(8192, C)
        in_maps.append(
            {
                "xs": slab_x,
                "xrw": xr,
                "wqkv": wqkv.astype(bfnp),
                "wp": wp.astype(bfnp),
                "ident": ident,
                "e128": e128,
                "e64": e64,
            }
        )
    return in_maps


def kernel(**inputs):
    global _CACHED, LAST_RESULTS
    if _CACHED is None:
        _CACHED = _build_program()
    nc = _CACHED
    in_maps = _prep_host(inputs)
    res = run_bass_kernel_spmd(
        nc,
        in_maps,
        list(range(NCORES)),
        trace=bool(int(os.environ.get("KTRACE", "0"))),
    )
    LAST_RESULTS = res
    out = np.empty((1, H * W, C), np.float32)
    og = out[0].reshape(H, W, C)
    for c in range(NCORES):
        # un-permute window-major [2, 16 wc, 16 r, 16 cc, C] -> rows/cols
        o = res.results[c]["out"].reshape(2, 16, 16, 16, C)
        og[32 * c : 32 * c + 32] = o.transpose(0, 2, 1, 3, 4).reshape(32, W, C)
    return out


# revision 26
# speedup vs baseline: 1.9788x; 1.0371x over previous
"""OCAB (overlapping cross-attention block) Trainium2 Bass kernel, v2.

Full inputs in, full outputs out; internally shards the B*nW window axis
across 8 NeuronCores (each core owns 2 window-rows = 32 image rows, with a
4-row halo for the overlapping k/v windows).

Per core:
  Phase 1+2 (fused, 21 groups of 512 tokens): stream x, LayerNorm
  (stats on vector, normalize on scalar Identity act with per-partition
  scale/bias), PE-transpose to channel-major group tiles (ones channel at
  row 180 rides the transposes), then 5 projection passes per group into
  persistent transposed slabs: q h0-3, k h0-3, v h0-3, (q45|k45) stacked,
  v h4-5. Biases ride as an extra input channel; LN gamma/beta and the
  attention scale are folded into the weights on host.

  Phase 3 (32 windows, software-pipelined): per window materialize
  contiguous k^T/v^T windows (vector copies), build token-major v tiles
  with DMA transposes, compute S^T per head with row-packed matmuls into
  a double-buffered 3-bank PSUM ring, exp on scalar (one act per key
  chunk), chunk-major attn@V accumulation (denominators ride as ones
  columns of v), denominator broadcast via constant matmuls, fast approx
  reciprocal, projection, residual add, contiguous window-major DRAM IO
  (host pre/post-permutes the window order).
"""

import os
import sys
from contextlib import ExitStack

import numpy as np
import ml_dtypes

for _p in ("/opt/trn_rl_repo", "/root/.axon_site/_ro/trn_rl_repo"):
    if os.path.isdir(_p) and _p not in sys.path:
        sys.path.append(_p)

import concourse.bass as bass
import concourse.tile as tile
from concourse import bacc, mybir
from concourse.bass_utils import run_bass_kernel_spmd

BF16 = mybir.dt.bfloat16
F32 = mybir.dt.float32
F32R = mybir.dt.float32r
bfnp = ml_dtypes.bfloat16
AF = mybir.ActivationFunctionType
ALU = mybir.AluOpType

# ---- problem constants (hardcoded per contract) ----
C = 180
NH = 6
HD = 30
WS = 16
OWS = 24
PADW = 4
H = W = 256
EPS = 1e-5
NCORES = 8

# ---- per-core slab geometry ----
RS = 40                    # slab image rows (32 + 2*4 halo)
CS = 264                   # slab image cols (256 + 2*4 zero pad)
TS_REAL = RS * CS          # 10560 real slab tokens
TCH = 84                   # token chunks of 128
T = TCH * 128              # 10752 padded slab tokens
NG = 21                    # 512-token groups
NWIN = 32                  # windows per core (2 window-rows x 16)

KOFF = [0, 128, 256, 384, 512]     # key chunk offsets
KC = [128, 128, 128, 128, 64]      # keys per chunk
# head -> column in es sbuf
EC = {0: 0, 4: 256, 1: 512, 5: 768, 2: 1024, 3: 1280}
# head -> column in S^T psum: h0/h4/h1/h5 in st_a, h2/h3 in st_b
STC = {0: 0, 4: 256, 1: 512, 5: 768, 2: 0, 3: 512}
# head -> PE row group (must equal stationary base partition)
TPR = {0: 0, 1: 32, 2: 64, 3: 96, 4: 0, 5: 32}
STW = [[0, 1, 2], [4, 5, 3]]       # S^T emission waves (distinct banks)
# head -> column block in token-major vw tiles
VC = {0: 0, 1: 32, 2: 64, 3: 96, 4: 128, 5: 160}
# attn@V jobs: (head, av row offset, av col offset)
AVJ = [(0, 0, 0), (1, 32, 0), (2, 64, 0), (3, 96, 0), (4, 0, 256), (5, 32, 256)]

LAST_RESULTS = None
_CACHED = None


def _build_program():
    nc = bacc.Bacc("TRN2", target_bir_lowering=False)

    xs_d = nc.declare_dram_parameter("xs", [T, C], F32, isOutput=False)
    xrw_d = nc.declare_dram_parameter("xrw", [8192, C], F32, isOutput=False)
    wqkv_d = nc.declare_dram_parameter("wqkv", [181, 576], BF16, isOutput=False)
    wp_d = nc.declare_dram_parameter("wp", [192, C], BF16, isOutput=False)
    id_d = nc.declare_dram_parameter("ident", [128, 128], BF16, isOutput=False)
    e128_d = nc.declare_dram_parameter("e128", [128, 128], F32R, isOutput=False)
    e64_d = nc.declare_dram_parameter("e64", [64, 64], F32R, isOutput=False)
    out_d = nc.declare_dram_parameter("out", [8192, C], F32, isOutput=True)
    dbg_d = nc.declare_dram_parameter("dbg", [128, 2048], F32, isOutput=True)
    kstage = os.environ.get("KSTAGE", "")

    with ExitStack() as ctx:
        tc = ctx.enter_context(tile.TileContext(nc))

        wp_pool = ctx.enter_context(tc.tile_pool(name="wts", bufs=1))
        WQKV0 = wp_pool.tile([128, 576], BF16, tag="wqkv0")
        WQKV1 = wp_pool.tile([53, 576], BF16, tag="wqkv1")
        WP0 = wp_pool.tile([128, C], BF16, tag="wp0")
        WP1 = wp_pool.tile([64, C], BF16, tag="wp1")
        IDT = wp_pool.tile([128, 128], BF16, tag="id")
        E128 = wp_pool.tile([128, 128], F32R, tag="e128")
        E64 = wp_pool.tile([64, 64], F32R, tag="e64")

        nc.sync.dma_start(WQKV0[:], wqkv_d[0:128, :])
        nc.sync.dma_start(WQKV1[:], wqkv_d[128:181, :])
        nc.sync.dma_start(WP0[:], wp_d[0:128, :])
        nc.sync.dma_start(WP1[:], wp_d[128:192, :])
        nc.sync.dma_start(IDT[:], id_d[:, :])
        nc.sync.dma_start(E128[:], e128_d[:, :])
        nc.sync.dma_start(E64[:], e64_d[:, :])

        # persistent transposed slabs
        slab = ctx.enter_context(tc.tile_pool(name="slab", bufs=1))
        Q03 = slab.tile([128, T], BF16, tag="q03")
        K03 = slab.tile([128, T], BF16, tag="k03")
        V03 = slab.tile([128, T], BF16, tag="v03")
        QK45 = slab.tile([128, T], BF16, tag="qk45")   # rows 0:64 q45, 64:128 k45
        V45 = slab.tile([64, T], BF16, tag="v45")
        SLABS = [Q03, K03, V03, QK45, V45]
        SLAB_ROWS = [128, 128, 128, 128, 64]
        # which engine copies each pass's psum to its slab
        COPY_ENG = ["scalar", "scalar", "scalar", "vector", "vector"]

        # ================= phase 1+2: LN + transpose + projections ==========
        with ExitStack() as pctx:
            p_x = pctx.enter_context(tc.tile_pool(name="p_x", bufs=6))
            p_sm = pctx.enter_context(tc.tile_pool(name="p_sm", bufs=6))
            p_tp = pctx.enter_context(tc.tile_pool(name="p_tp", bufs=2, space="PSUM"))
            p_qp = pctx.enter_context(tc.tile_pool(name="p_qp", bufs=3, space="PSUM"))
            p_xtg = pctx.enter_context(tc.tile_pool(name="p_xtg", bufs=3))

            for g in range(NG):
                tp = p_tp.tile([128, 512], BF16, tag="tp")
                tp2 = p_tp.tile([128, 512], BF16, tag="tp2")
                # batch each LN stage over the group's 4 chunks so the
                # cross-engine chain (vector->gpsimd->scalar->vector) overlaps
                xts, aggrs, rstds, nbs = [], [], [], []
                for j in range(4):
                    tch = 4 * g + j
                    xt = p_x.tile([128, C], F32, tag="x", name=f"x{tch}")
                    nc.sync.dma_start(xt[:], xs_d[128 * tch : 128 * (tch + 1), :])
                    stats = p_sm.tile([128, 6], F32, tag="st", name=f"st{tch}")
                    aggr = p_sm.tile([128, 2], F32, tag="ag", name=f"ag{tch}")
                    nc.vector.bn_stats(stats[:], xt[:])
                    nc.vector.bn_aggr(aggr[:], stats[:])
                    xts.append(xt)
                    aggrs.append(aggr)
                for j in range(4):
                    vpe = p_sm.tile([128, 1], F32, tag="vpe", name=f"vpe{g}_{j}")
                    nc.gpsimd.tensor_scalar_add(vpe[:], aggrs[j][:, 1:2], EPS)
                    sd = p_sm.tile([128, 1], F32, tag="sd", name=f"sd{g}_{j}")
                    nc.scalar.activation(sd[:], vpe[:], AF.Sqrt, bias=0.0)
                    rstd = p_sm.tile([128, 1], F32, tag="rstd", name=f"rs{g}_{j}")
                    nc.vector.reciprocal(rstd[:], sd[:])
                    nbias = p_sm.tile([128, 1], F32, tag="nb", name=f"nb{g}_{j}")
                    nc.vector.scalar_tensor_tensor(
                        nbias[:], aggrs[j][:, 0:1], -1.0, rstd[:],
                        op0=ALU.mult, op1=ALU.mult,
                    )
                    rstds.append(rstd)
                    nbs.append(nbias)
                for j in range(4):
                    xn = p_x.tile([128, 256], BF16, tag="xn", name=f"xn{g}_{j}")
                    nc.scalar.activation(
                        xn[:, 0:C], xts[j][:], AF.Identity,
                        bias=nbs[j][:], scale=rstds[j][:],
                    )
                    nc.gpsimd.memset(xn[:, C : C + 1], 1.0)
                    nc.tensor.transpose(
                        tp[:, 128 * j : 128 * (j + 1)], xn[:, 0:128], IDT[:]
                    )
                    nc.tensor.transpose(
                        tp2[:, 128 * j : 128 * (j + 1)], xn[:, 128:256], IDT[:]
                    )
                xt0g = p_xtg.tile([128, 512], BF16, tag="xt0g")
                xt1g = p_xtg.tile([128, 512], BF16, tag="xt1g")
                nc.vector.tensor_copy(xt0g[:], tp[:])
                nc.vector.tensor_copy(xt1g[0:53, :], tp2[0:53, :])

                for p in range(5):
                    c0, c1 = (128 * p, 128 * p + 128) if p < 4 else (512, 576)
                    outw = SLAB_ROWS[p]
                    qp = p_qp.tile([128, 512], F32, tag="qp")
                    nc.tensor.matmul(
                        qp[0:outw, :], WQKV0[:, c0:c1], xt0g[:],
                        start=True, stop=False,
                    )
                    nc.tensor.matmul(
                        qp[0:outw, :], WQKV1[:, c0:c1], xt1g[0:53, :],
                        start=False, stop=True,
                    )
                    dst = SLABS[p][0:outw, 512 * g : 512 * (g + 1)]
                    if COPY_ENG[p] == "scalar":
                        nc.scalar.copy(dst, qp[0:outw, :])
                    else:
                        nc.vector.tensor_copy(dst, qp[0:outw, :])

        # ================= phase 3: windowed attention =======================
        with ExitStack() as actx:
            a_sta = actx.enter_context(tc.tile_pool(name="a_sta", bufs=2, space="PSUM"))
            a_stb = actx.enter_context(tc.tile_pool(name="a_stb", bufs=1, space="PSUM"))
            a_ep = actx.enter_context(tc.tile_pool(name="a_ep", bufs=1, space="PSUM"))
            a_avp = actx.enter_context(tc.tile_pool(name="a_avp", bufs=1, space="PSUM"))
            a_kv = actx.enter_context(tc.tile_pool(name="a_kv", bufs=2))
            a_vw = actx.enter_context(tc.tile_pool(name="a_vw", bufs=2))
            a_es = actx.enter_context(tc.tile_pool(name="a_es", bufs=7))
            a_sb = actx.enter_context(tc.tile_pool(name="a_sb", bufs=2))

            q03_pat = Q03[:, 0:TS_REAL].rearrange("p (r c) -> p r c", c=CS)
            k03_pat = K03[:, 0:TS_REAL].rearrange("p (r c) -> p r c", c=CS)
            v03_pat = V03[:, 0:TS_REAL].rearrange("p (r c) -> p r c", c=CS)
            qk45_pat = QK45[:, 0:TS_REAL].rearrange("p (r c) -> p r c", c=CS)
            v45_pat = V45[:, 0:TS_REAL].rearrange("p (r c) -> p r c", c=CS)

            # per-window state carried across pipeline stages
            wstate = {}

            # stage window 0's k^T/v^T slices ahead of the loop
            kw0n = a_kv.tile([128, 576], BF16, tag="kw0", name="kw0_p")
            kw1n = a_kv.tile([64, 576], BF16, tag="kw1", name="kw1_p")
            vwT0n = a_kv.tile([128, 576], BF16, tag="vwT0", name="vwT0_p")
            vwT1n = a_kv.tile([64, 576], BF16, tag="vwT1", name="vwT1_p")
            nc.vector.tensor_copy(
                kw0n[:].rearrange("p (r c) -> p r c", c=OWS),
                k03_pat[:, 0:OWS, 0:OWS],
            )
            nc.vector.tensor_copy(
                kw1n[:].rearrange("p (r c) -> p r c", c=OWS),
                qk45_pat[64:128, 0:OWS, 0:OWS],
            )
            nc.vector.tensor_copy(
                vwT0n[:].rearrange("p (r c) -> p r c", c=OWS),
                v03_pat[:, 0:OWS, 0:OWS],
            )
            nc.vector.tensor_copy(
                vwT1n[:].rearrange("p (r c) -> p r c", c=OWS),
                v45_pat[:, 0:OWS, 0:OWS],
            )
            kvt_next = (kw0n, kw1n, vwT0n, vwT1n)

            for w in range(NWIN + 2):
                cur = w < NWIN
                prev = 1 <= w <= NWIN
                if cur:
                    wrl, wc = w // 16, w % 16
                    r0, c0 = WS * wrl, WS * wc
                if prev:
                    st_prev = wstate[w - 1]

                # ---- 1. denominator broadcast for window w-1 ----
                if prev:
                    P = st_prev
                    ex = a_ep.tile([128, 512], F32, tag="ep", name=f"ex{w}")
                    nc.tensor.matmul(
                        ex[:, 0:256], E128[:], P["rsb"][:, 0:256],
                        start=True, stop=True,
                    )
                    nc.tensor.matmul(
                        ex[0:64, 256:512], E64[:], P["rsb"][0:64, 256:512],
                        start=True, stop=True,
                    )

                # ---- 4. normalize window w-1 (vector, early: unblocks PE) ----
                if prev:
                    P = st_prev
                    exsb = a_sb.tile([128, 512], F32, tag="exsb", name=f"exsb{w}")
                    if int(os.environ.get("KRECIP", "1")):
                        nc.vector.reciprocal_approx_fast(exsb[:, 0:256], ex[:, 0:256])
                        nc.vector.reciprocal_approx_fast(
                            exsb[0:64, 256:512], ex[0:64, 256:512]
                        )
                    else:
                        nc.vector.reciprocal(exsb[:, 0:256], ex[:, 0:256])
                        nc.vector.reciprocal(exsb[0:64, 256:512], ex[0:64, 256:512])
                    att = a_sb.tile([128, 512], BF16, tag="att", name=f"att{w}")
                    nc.vector.tensor_tensor(
                        att[:, 0:256], P["av"][:, 0:256], exsb[:, 0:256],
                        op=ALU.mult,
                    )
                    nc.vector.tensor_tensor(
                        att[0:64, 256:512], P["av"][0:64, 256:512],
                        exsb[0:64, 256:512], op=ALU.mult,
                    )

                # ---- 3. xres load; token-major v tiles via PE transpose ----
                if prev:
                    xres = a_sb.tile([128, 360], F32, tag="xres", name=f"xres{w}")
                    nc.sync.dma_start(
                        xres[:].rearrange("p (j d) -> p j d", d=C),
                        xrw_d[256 * (w - 1) : 256 * w, :].rearrange(
                            "(j p) d -> p j d", p=128
                        ),
                    )
                if cur:
                    kw0, kw1, vwT0, vwT1 = kvt_next
                    vwp = a_ep.tile([128, 960], BF16, tag="ep", name=f"vwp{w}")
                    vw = [
                        a_vw.tile([128, 192], BF16, tag=f"vw{i}", name=f"vw{w}_{i}")
                        for i in range(5)
                    ]
                    for i, off in enumerate(KOFF):
                        kc = KC[i]
                        nc.tensor.transpose(
                            vwp[0:kc, 192 * i : 192 * i + 128],
                            vwT0[:, off : off + kc],
                            IDT[:],
                        )
                        nc.tensor.transpose(
                            vwp[0:kc, 192 * i + 128 : 192 * i + 192],
                            vwT1[:, off : off + kc],
                            IDT[0:64, 0:64],
                        )
                        nc.vector.tensor_copy(
                            vw[i][0:kc, :], vwp[0:kc, 192 * i : 192 * i + 192]
                        )

                # ---- 3b. stage k^T/v^T window slices for window w+1 ----
                if w + 1 < NWIN:
                    nwrl, nwc = (w + 1) // 16, (w + 1) % 16
                    nr0, nc0 = WS * nwrl, WS * nwc
                    kw0n = a_kv.tile([128, 576], BF16, tag="kw0", name=f"kw0_{w+1}")
                    kw1n = a_kv.tile([64, 576], BF16, tag="kw1", name=f"kw1_{w+1}")
                    vwT0n = a_kv.tile([128, 576], BF16, tag="vwT0", name=f"vwT0_{w+1}")
                    vwT1n = a_kv.tile([64, 576], BF16, tag="vwT1", name=f"vwT1_{w+1}")
                    nc.vector.tensor_copy(
                        kw0n[:].rearrange("p (r c) -> p r c", c=OWS),
                        k03_pat[:, nr0 : nr0 + OWS, nc0 : nc0 + OWS],
                    )
                    nc.vector.tensor_copy(
                        kw1n[:].rearrange("p (r c) -> p r c", c=OWS),
                        qk45_pat[64:128, nr0 : nr0 + OWS, nc0 : nc0 + OWS],
                    )
                    nc.vector.tensor_copy(
                        vwT0n[:].rearrange("p (r c) -> p r c", c=OWS),
                        v03_pat[:, nr0 : nr0 + OWS, nc0 : nc0 + OWS],
                    )
                    nc.vector.tensor_copy(
                        vwT1n[:].rearrange("p (r c) -> p r c", c=OWS),
                        v45_pat[:, nr0 : nr0 + OWS, nc0 : nc0 + OWS],
                    )
                    kvt_next = (kw0n, kw1n, vwT0n, vwT1n)

                # ---- 5. S^T + exp + attn@V for window w ----
                if cur:
                    av = a_avp.tile([128, 512], F32, tag="av", name=f"av{w}")
                    es_list = []

                    def st_chunk(ch):
                        off, kc = KOFF[ch], KC[ch]
                        sta = a_sta.tile(
                            [128, 1024], F32, tag="sta", name=f"sta{w}_{ch}"
                        )
                        stb = a_stb.tile(
                            [128, 1024], F32, tag="stb", name=f"stb{w}_{ch}"
                        )
                        for wave in STW:
                            for h in wave:
                                if h < 4:
                                    ktile, qtile = kw0, q03_pat
                                    kr, qr = 32 * h, 32 * h
                                else:
                                    ktile, qtile = kw1, qk45_pat
                                    kr, qr = 32 * (h - 4), 32 * (h - 4)
                                st = sta if h in (0, 1, 4, 5) else stb
                                nc.tensor.matmul(
                                    st[0:kc, STC[h] : STC[h] + 256],
                                    ktile[kr : kr + 32, off : off + kc],
                                    qtile[
                                        qr : qr + 32,
                                        PADW + r0 : PADW + r0 + WS,
                                        PADW + c0 : PADW + c0 + WS,
                                    ],
                                    start=True, stop=True,
                                    tile_position=(TPR[h], 0),
                                )
                        return sta, stb

                    def exp_chunk(ch, sta, stb):
                        kc = KC[ch]
                        es = a_es.tile(
                            [128, 1536], BF16, tag="es", name=f"es{w}_{ch}"
                        )
                        nc.scalar.activation(
                            es[0:kc, 0:1024], sta[0:kc, :], AF.Exp
                        )
                        nc.scalar.activation(
                            es[0:kc, 1024:1536].rearrange(
                                "p (a b) -> p a b", b=256
                            ),
                            stb[0:kc, :].rearrange("p (a b) -> p a b", b=512)[
                                :, :, 0:256
                            ],
                            AF.Exp,
                        )
                        es_list.append(es)

                    def av_one(h, ro, co, ch):
                        # jobs with disjoint psum partitions may hold
                        # concurrently-open accumulation groups; jobs sharing
                        # partitions+bank (h0/h4, h1/h5) must not interleave
                        kc = KC[ch]
                        nc.tensor.matmul(
                            av[ro : ro + 32, co : co + 256],
                            vw[ch][0:kc, VC[h] : VC[h] + 32],
                            es_list[ch][0:kc, EC[h] : EC[h] + 256],
                            start=(ch == 0), stop=(ch == 4),
                            tile_position=(0, ro),
                            skip_group_check=True,
                        )

                    def av_chunk(ch):
                        for h, ro, co in AVJ[:4]:
                            av_one(h, ro, co, ch)

                    sts = [st_chunk(0)]
                    exp_chunk(0, *sts[0])
                    sts.append(st_chunk(1))
                    exp_chunk(1, *sts[1])
                    av_chunk(0)
                    sts.append(st_chunk(2))
                    exp_chunk(2, *sts[2])
                    av_chunk(1)
                    sts.append(st_chunk(3))
                    exp_chunk(3, *sts[3])
                    av_chunk(2)
                    sts.append(st_chunk(4))
                    exp_chunk(4, *sts[4])
                    av_chunk(3)
                    av_chunk(4)
                    for ch in range(5):
                        for h, ro, co in AVJ[4:]:
                            av_one(h, ro, co, ch)

                # ---- 6. projection + residual + store for window w-1 ----
                if prev:
                    P = st_prev
                    pp = a_ep.tile([128, 360], F32, tag="ep", name=f"pp{w}")
                    for qc in range(2):
                        nc.tensor.matmul(
                            pp[:, 180 * qc : 180 * qc + 180],
                            att[:, 128 * qc : 128 * (qc + 1)],
                            WP0[:],
                            start=True, stop=False,
                        )
                        nc.tensor.matmul(
                            pp[:, 180 * qc : 180 * qc + 180],
                            att[0:64, 256 + 128 * qc : 256 + 128 * (qc + 1)],
                            WP1[:],
                            start=False, stop=True,
                        )
                    ot = a_sb.tile([128, 360], F32, tag="ot", name=f"ot{w}", bufs=3)
                    nc.vector.tensor_tensor(ot[:], pp[:], xres[:], op=ALU.add)
                    st_prev["ot"] = ot

                # ---- 6b. store window w-2 (data long ready: no queue block)
                if 2 <= w <= NWIN + 1:
                    nc.sync.dma_start(
                        out_d[256 * (w - 2) : 256 * (w - 1), :].rearrange(
                            "(j p) d -> p j d", p=128
                        ),
                        wstate[w - 2]["ot"][:].rearrange("p (j d) -> p j d", d=C),
                    )

                # ---- debug dumps for window 0 ----
                if cur and w == 0 and kstage:
                    dbg = a_sb.tile([128, 2048], F32, tag="dbg", bufs=1)
                    if kstage == "q":
                        nc.vector.tensor_copy(dbg[:, 0:2048], Q03[:, 0:2048])
                    elif kstage == "kw":
                        nc.vector.tensor_copy(dbg[:, 0:576], kw0[:])
                        nc.vector.tensor_copy(dbg[0:64, 576:1152], kw1[:])
                    elif kstage == "vw0":
                        nc.vector.tensor_copy(dbg[:, 0:192], vw[0][:])
                        nc.vector.tensor_copy(dbg[:, 192:384], vw[4][:])
                    elif kstage == "es0":
                        nc.vector.tensor_copy(dbg[:, 0:1536], es_list[0][:])
                    elif kstage == "es4":
                        nc.vector.tensor_copy(dbg[0:64, 0:1536], es_list[4][0:64, :])
                    elif kstage == "av":
                        nc.vector.tensor_copy(dbg[:, 0:512], av[:])
                    nc.sync.dma_start(dbg_d[:, :], dbg[:])

                # ---- 7. rowsum snapshot for window w ----
                if cur:
                    rsb = a_sb.tile([128, 512], F32R, tag="rsb", name=f"rsb{w}")
                    nc.vector.tensor_copy(rsb[:], av[:])
                    wstate[w] = {"av": av, "rsb": rsb}
                    wstate.pop(w - 3, None)

    nc.compile()
    return nc


def _prep_host(inputs):
    x = np.ascontiguousarray(inputs["x"], dtype=np.float32)[0]  # [65536, 180]
    norm_w = np.asarray(inputs["norm_w"], np.float32)
    norm_b = np.asarray(inputs["norm_b"], np.float32)
    q_w = np.asarray(inputs["q_w"], np.float32)
    q_b = np.asarray(inputs["q_b"], np.float32)
    kv_w = np.asarray(inputs["kv_w"], np.float32)
    kv_b = np.asarray(inputs["kv_b"], np.float32)
    proj_w = np.asarray(inputs["proj_w"], np.float32)
    proj_b = np.asarray(inputs["proj_b"], np.float32)

    scale = HD ** -0.5
    Wq = norm_w[:, None] * q_w * scale
    bq = (norm_b @ q_w + q_b) * scale
    Wk = norm_w[:, None] * kv_w[:, :C]
    bk = norm_b @ kv_w[:, :C] + kv_b[:C]
    Wv = norm_w[:, None] * kv_w[:, C:]
    bv = norm_b @ kv_w[:, C:] + kv_b[C:]

    # wqkv [181, 576]: q03 | k03 | v03 | (q45|k45) | v45, 32-col head blocks;
    # v blocks carry 1.0 at row 180 in cols 30/31 (denominator ride-along)
    wqkv = np.zeros((181, 576), np.float32)

    def put(colbase, h, Wm, bm, ones):
        col = colbase + 32 * (h % 4)
        wqkv[:C, col : col + HD] = Wm[:, HD * h : HD * (h + 1)]
        wqkv[C, col : col + HD] = bm[HD * h : HD * (h + 1)]
        if ones:
            wqkv[C, col + 30] = 1.0
            wqkv[C, col + 31] = 1.0

    for h in range(4):
        put(0, h, Wq, bq, False)
        put(128, h, Wk, bk, False)
        put(256, h, Wv, bv, True)
    for h in (4, 5):
        put(384, h, Wq, bq, False)
        put(448, h, Wk, bk, False)
        put(512, h, Wv, bv, True)

    # wp [192, 180]: rows 0:128 = proj rows h0-3 (32-blocks), 128:192 h4-5
    wp = np.zeros((192, C), np.float32)
    for h in range(NH):
        row = 32 * h if h < 4 else 128 + 32 * (h - 4)
        wp[row : row + HD, :] = proj_w[HD * h : HD * (h + 1), :]

    ident = np.eye(128, dtype=bfnp)
    e128 = np.zeros((128, 128), np.float32)
    for j in range(4):
        e128[32 * j + 30, 32 * j : 32 * j + 32] = 1.0
    e64 = np.zeros((64, 64), np.float32)
    e64[30, 0:32] = 1.0
    e64[62, 32:64] = 1.0

    # per-core slabs
    xg = x.reshape(H, W, C)
    xpad = np.zeros((H + 2 * PADW, CS, C), np.float32)
    xpad[PADW : PADW + H, PADW : PADW + W, :] = xg
    xres_full = (x + proj_b).reshape(H, W, C)

    in_maps = []
    for c in range(NCORES):
        slab_x = np.zeros((T, C), np.float32)
        slab_x[:TS_REAL] = xpad[32 * c : 32 * c + RS].reshape(TS_REAL, C)
        # window-major residual: [2 wrl, 16 r, 16 wc, 16 cc, C] ->
        # (wrl, wc, r, cc)
        xr = xres_full[32 * c : 32 * c + 32].reshape(2, 16, 16, 16, C)
        xr = np.ascontiguousarray(xr.transpose(0, 2, 1, 3, 4)).reshape(8192, C)
        in_maps.append(
            {
                "xs": slab_x,
                "xrw": xr,
                "wqkv": wqkv.astype(bfnp),
                "wp": wp.astype(bfnp),
                "ident": ident,
                "e128": e128,
                "e64": e64,
            }
        )
    return in_maps


def kernel(**inputs):
    global _CACHED, LAST_RESULTS
    if _CACHED is None:
        _CACHED = _build_program()
    nc = _CACHED
    in_maps = _prep_host(inputs)
    res = run_bass_kernel_spmd(
        nc,
        in_maps,
        list(range(NCORES)),
        trace=bool(int(os.environ.get("KTRACE", "0"))),
    )
    LAST_RESULTS = res
    out = np.empty((1, H * W, C), np.float32)
    og = out[0].reshape(H, W, C)
    for c in range(NCORES):
        # un-permute window-major [2, 16 wc, 16 r, 16 cc, C] -> rows/cols
        o = res.results[c]["out"].reshape(2, 16, 16, 16, C)
        og[32 * c : 32 * c + 32] = o.transpose(0, 2, 1, 3, 4).reshape(32, W, C)
    return out


# revision 27
# speedup vs baseline: 1.9828x; 1.0020x over previous
"""OCAB (overlapping cross-attention block) Trainium2 Bass kernel, v2.

Full inputs in, full outputs out; internally shards the B*nW window axis
across 8 NeuronCores (each core owns 2 window-rows = 32 image rows, with a
4-row halo for the overlapping k/v windows).

Per core:
  Phase 1+2 (fused, 21 groups of 512 tokens): stream x, LayerNorm
  (stats on vector, normalize on scalar Identity act with per-partition
  scale/bias), PE-transpose to channel-major group tiles (ones channel at
  row 180 rides the transposes), then 5 projection passes per group into
  persistent transposed slabs: q h0-3, k h0-3, v h0-3, (q45|k45) stacked,
  v h4-5. Biases ride as an extra input channel; LN gamma/beta and the
  attention scale are folded into the weights on host.

  Phase 3 (32 windows, software-pipelined): per window materialize
  contiguous k^T/v^T windows (vector copies), build token-major v tiles
  with DMA transposes, compute S^T per head with row-packed matmuls into
  a double-buffered 3-bank PSUM ring, exp on scalar (one act per key
  chunk), chunk-major attn@V accumulation (denominators ride as ones
  columns of v), denominator broadcast via constant matmuls, fast approx
  reciprocal, projection, residual add, contiguous window-major DRAM IO
  (host pre/post-permutes the window order).
"""

import os
import sys
from contextlib import ExitStack

import numpy as np
import ml_dtypes

for _p in ("/opt/trn_rl_repo", "/root/.axon_site/_ro/trn_rl_repo"):
    if os.path.isdir(_p) and _p not in sys.path:
        sys.path.append(_p)

import concourse.bass as bass
import concourse.tile as tile
from concourse import bacc, mybir
from concourse.bass_utils import run_bass_kernel_spmd

BF16 = mybir.dt.bfloat16
F32 = mybir.dt.float32
F32R = mybir.dt.float32r
bfnp = ml_dtypes.bfloat16
AF = mybir.ActivationFunctionType
ALU = mybir.AluOpType

# ---- problem constants (hardcoded per contract) ----
C = 180
NH = 6
HD = 30
WS = 16
OWS = 24
PADW = 4
H = W = 256
EPS = 1e-5
NCORES = 8

# ---- per-core slab geometry ----
RS = 40                    # slab image rows (32 + 2*4 halo)
CS = 264                   # slab image cols (256 + 2*4 zero pad)
TS_REAL = RS * CS          # 10560 real slab tokens
TCH = 84                   # token chunks of 128
T = TCH * 128              # 10752 padded slab tokens
NG = 21                    # 512-token groups
NWIN = 32                  # windows per core (2 window-rows x 16)

KOFF = [0, 128, 256, 384, 512]     # key chunk offsets
KC = [128, 128, 128, 128, 64]      # keys per chunk
# head -> column in es sbuf
EC = {0: 0, 4: 256, 1: 512, 5: 768, 2: 1024, 3: 1280}
# head -> column in S^T psum: h0/h4/h1/h5 in st_a, h2/h3 in st_b
STC = {0: 0, 4: 256, 1: 512, 5: 768, 2: 0, 3: 512}
# head -> PE row group (must equal stationary base partition)
TPR = {0: 0, 1: 32, 2: 64, 3: 96, 4: 0, 5: 32}
STW = [[0, 1, 2], [4, 5, 3]]       # S^T emission waves (distinct banks)
# head -> column block in token-major vw tiles
VC = {0: 0, 1: 32, 2: 64, 3: 96, 4: 128, 5: 160}
# attn@V jobs: (head, av row offset, av col offset)
AVJ = [(0, 0, 0), (1, 32, 0), (2, 64, 0), (3, 96, 0), (4, 0, 256), (5, 32, 256)]

LAST_RESULTS = None
_CACHED = None


def _build_program():
    nc = bacc.Bacc("TRN2", target_bir_lowering=False)

    xs_d = nc.declare_dram_parameter("xs", [T, C], F32, isOutput=False)
    xrw_d = nc.declare_dram_parameter("xrw", [8192, C], F32, isOutput=False)
    wqkv_d = nc.declare_dram_parameter("wqkv", [181, 576], BF16, isOutput=False)
    wp_d = nc.declare_dram_parameter("wp", [192, C], BF16, isOutput=False)
    id_d = nc.declare_dram_parameter("ident", [128, 128], BF16, isOutput=False)
    e128_d = nc.declare_dram_parameter("e128", [128, 128], F32R, isOutput=False)
    e64_d = nc.declare_dram_parameter("e64", [64, 64], F32R, isOutput=False)
    out_d = nc.declare_dram_parameter("out", [8192, C], F32, isOutput=True)
    dbg_d = nc.declare_dram_parameter("dbg", [128, 2048], F32, isOutput=True)
    kstage = os.environ.get("KSTAGE", "")

    with ExitStack() as ctx:
        tc = ctx.enter_context(tile.TileContext(nc))

        wp_pool = ctx.enter_context(tc.tile_pool(name="wts", bufs=1))
        WQKV0 = wp_pool.tile([128, 576], BF16, tag="wqkv0")
        WQKV1 = wp_pool.tile([53, 576], BF16, tag="wqkv1")
        WP0 = wp_pool.tile([128, C], BF16, tag="wp0")
        WP1 = wp_pool.tile([64, C], BF16, tag="wp1")
        IDT = wp_pool.tile([128, 128], BF16, tag="id")
        E128 = wp_pool.tile([128, 128], F32R, tag="e128")
        E64 = wp_pool.tile([64, 64], F32R, tag="e64")

        nc.sync.dma_start(WQKV0[:], wqkv_d[0:128, :])
        nc.sync.dma_start(WQKV1[:], wqkv_d[128:181, :])
        nc.sync.dma_start(WP0[:], wp_d[0:128, :])
        nc.sync.dma_start(WP1[:], wp_d[128:192, :])
        nc.sync.dma_start(IDT[:], id_d[:, :])
        nc.sync.dma_start(E128[:], e128_d[:, :])
        nc.sync.dma_start(E64[:], e64_d[:, :])

        # persistent transposed slabs
        slab = ctx.enter_context(tc.tile_pool(name="slab", bufs=1))
        Q03 = slab.tile([128, T], BF16, tag="q03")
        K03 = slab.tile([128, T], BF16, tag="k03")
        V03 = slab.tile([128, T], BF16, tag="v03")
        QK45 = slab.tile([128, T], BF16, tag="qk45")   # rows 0:64 q45, 64:128 k45
        V45 = slab.tile([64, T], BF16, tag="v45")
        SLABS = [Q03, K03, V03, QK45, V45]
        SLAB_ROWS = [128, 128, 128, 128, 64]
        # which engine copies each pass's psum to its slab
        COPY_ENG = ["scalar", "scalar", "scalar", "vector", "vector"]

        # ================= phase 1+2: LN + transpose + projections ==========
        with ExitStack() as pctx:
            p_x = pctx.enter_context(tc.tile_pool(name="p_x", bufs=6))
            p_sm = pctx.enter_context(tc.tile_pool(name="p_sm", bufs=6))
            p_tp = pctx.enter_context(tc.tile_pool(name="p_tp", bufs=2, space="PSUM"))
            p_qp = pctx.enter_context(tc.tile_pool(name="p_qp", bufs=3, space="PSUM"))
            p_xtg = pctx.enter_context(tc.tile_pool(name="p_xtg", bufs=3))

            for g in range(NG):
                tp = p_tp.tile([128, 512], BF16, tag="tp")
                tp2 = p_tp.tile([128, 512], BF16, tag="tp2")
                # batch each LN stage over the group's 4 chunks so the
                # cross-engine chain (vector->gpsimd->scalar->vector) overlaps
                xts, aggrs, rstds, nbs = [], [], [], []
                for j in range(4):
                    tch = 4 * g + j
                    xt = p_x.tile([128, C], F32, tag="x", name=f"x{tch}")
                    nc.sync.dma_start(xt[:], xs_d[128 * tch : 128 * (tch + 1), :])
                    stats = p_sm.tile([128, 6], F32, tag="st", name=f"st{tch}")
                    aggr = p_sm.tile([128, 2], F32, tag="ag", name=f"ag{tch}")
                    nc.vector.bn_stats(stats[:], xt[:])
                    nc.vector.bn_aggr(aggr[:], stats[:])
                    xts.append(xt)
                    aggrs.append(aggr)
                for j in range(4):
                    vpe = p_sm.tile([128, 1], F32, tag="vpe", name=f"vpe{g}_{j}")
                    nc.gpsimd.tensor_scalar_add(vpe[:], aggrs[j][:, 1:2], EPS)
                    sd = p_sm.tile([128, 1], F32, tag="sd", name=f"sd{g}_{j}")
                    nc.scalar.activation(sd[:], vpe[:], AF.Sqrt, bias=0.0)
                    rstd = p_sm.tile([128, 1], F32, tag="rstd", name=f"rs{g}_{j}")
                    nc.vector.reciprocal(rstd[:], sd[:])
                    nbias = p_sm.tile([128, 1], F32, tag="nb", name=f"nb{g}_{j}")
                    nc.vector.scalar_tensor_tensor(
                        nbias[:], aggrs[j][:, 0:1], -1.0, rstd[:],
                        op0=ALU.mult, op1=ALU.mult,
                    )
                    rstds.append(rstd)
                    nbs.append(nbias)
                for j in range(4):
                    xn = p_x.tile([128, 256], BF16, tag="xn", name=f"xn{g}_{j}")
                    nc.scalar.activation(
                        xn[:, 0:C], xts[j][:], AF.Identity,
                        bias=nbs[j][:], scale=rstds[j][:],
                    )
                    nc.gpsimd.memset(xn[:, C : C + 1], 1.0)
                    nc.tensor.transpose(
                        tp[:, 128 * j : 128 * (j + 1)], xn[:, 0:128], IDT[:]
                    )
                    nc.tensor.transpose(
                        tp2[:, 128 * j : 128 * (j + 1)], xn[:, 128:256], IDT[:]
                    )
                xt0g = p_xtg.tile([128, 512], BF16, tag="xt0g")
                xt1g = p_xtg.tile([128, 512], BF16, tag="xt1g")
                nc.vector.tensor_copy(xt0g[:], tp[:])
                nc.vector.tensor_copy(xt1g[0:53, :], tp2[0:53, :])

                for p in range(5):
                    c0, c1 = (128 * p, 128 * p + 128) if p < 4 else (512, 576)
                    outw = SLAB_ROWS[p]
                    qp = p_qp.tile([128, 512], F32, tag="qp")
                    nc.tensor.matmul(
                        qp[0:outw, :], WQKV0[:, c0:c1], xt0g[:],
                        start=True, stop=False,
                    )
                    nc.tensor.matmul(
                        qp[0:outw, :], WQKV1[:, c0:c1], xt1g[0:53, :],
                        start=False, stop=True,
                    )
                    dst = SLABS[p][0:outw, 512 * g : 512 * (g + 1)]
                    if COPY_ENG[p] == "scalar":
                        nc.scalar.copy(dst, qp[0:outw, :])
                    else:
                        nc.vector.tensor_copy(dst, qp[0:outw, :])

        # ================= phase 3: windowed attention =======================
        with ExitStack() as actx:
            a_sta = actx.enter_context(tc.tile_pool(name="a_sta", bufs=2, space="PSUM"))
            a_stb = actx.enter_context(tc.tile_pool(name="a_stb", bufs=1, space="PSUM"))
            a_ep = actx.enter_context(tc.tile_pool(name="a_ep", bufs=1, space="PSUM"))
            a_avp = actx.enter_context(tc.tile_pool(name="a_avp", bufs=1, space="PSUM"))
            a_kv = actx.enter_context(tc.tile_pool(name="a_kv", bufs=2))
            a_vw = actx.enter_context(tc.tile_pool(name="a_vw", bufs=2))
            a_es = actx.enter_context(tc.tile_pool(name="a_es", bufs=7))
            a_sb = actx.enter_context(tc.tile_pool(name="a_sb", bufs=2))

            q03_pat = Q03[:, 0:TS_REAL].rearrange("p (r c) -> p r c", c=CS)
            k03_pat = K03[:, 0:TS_REAL].rearrange("p (r c) -> p r c", c=CS)
            v03_pat = V03[:, 0:TS_REAL].rearrange("p (r c) -> p r c", c=CS)
            qk45_pat = QK45[:, 0:TS_REAL].rearrange("p (r c) -> p r c", c=CS)
            v45_pat = V45[:, 0:TS_REAL].rearrange("p (r c) -> p r c", c=CS)

            # per-window state carried across pipeline stages
            wstate = {}

            # stage window 0's k^T/v^T slices ahead of the loop
            kw0n = a_kv.tile([128, 576], BF16, tag="kw0", name="kw0_p")
            kw1n = a_kv.tile([64, 576], BF16, tag="kw1", name="kw1_p")
            vwT0n = a_kv.tile([128, 576], BF16, tag="vwT0", name="vwT0_p")
            vwT1n = a_kv.tile([64, 576], BF16, tag="vwT1", name="vwT1_p")
            nc.vector.tensor_copy(
                kw0n[:].rearrange("p (r c) -> p r c", c=OWS),
                k03_pat[:, 0:OWS, 0:OWS],
            )
            nc.vector.tensor_copy(
                kw1n[:].rearrange("p (r c) -> p r c", c=OWS),
                qk45_pat[64:128, 0:OWS, 0:OWS],
            )
            nc.vector.tensor_copy(
                vwT0n[:].rearrange("p (r c) -> p r c", c=OWS),
                v03_pat[:, 0:OWS, 0:OWS],
            )
            nc.vector.tensor_copy(
                vwT1n[:].rearrange("p (r c) -> p r c", c=OWS),
                v45_pat[:, 0:OWS, 0:OWS],
            )
            kvt_next = (kw0n, kw1n, vwT0n, vwT1n)

            for w in range(NWIN + 2):
                cur = w < NWIN
                prev = 1 <= w <= NWIN
                if cur:
                    wrl, wc = w // 16, w % 16
                    r0, c0 = WS * wrl, WS * wc
                if prev:
                    st_prev = wstate[w - 1]

                # ---- 1. denominator broadcast for window w-1 ----
                if prev:
                    P = st_prev
                    ex = a_ep.tile([128, 512], F32, tag="ep", name=f"ex{w}")
                    nc.tensor.matmul(
                        ex[:, 0:256], E128[:], P["rsb"][:, 0:256],
                        start=True, stop=True,
                    )
                    nc.tensor.matmul(
                        ex[0:64, 256:512], E64[:], P["rsb"][0:64, 256:512],
                        start=True, stop=True,
                    )

                # ---- 4. normalize window w-1 (vector, early: unblocks PE) ----
                if prev:
                    P = st_prev
                    exsb = a_sb.tile([128, 512], F32, tag="exsb", name=f"exsb{w}")
                    if int(os.environ.get("KRECIP", "1")):
                        nc.vector.reciprocal_approx_fast(exsb[:, 0:256], ex[:, 0:256])
                        nc.vector.reciprocal_approx_fast(
                            exsb[0:64, 256:512], ex[0:64, 256:512]
                        )
                    else:
                        nc.vector.reciprocal(exsb[:, 0:256], ex[:, 0:256])
                        nc.vector.reciprocal(exsb[0:64, 256:512], ex[0:64, 256:512])
                    att = a_sb.tile([128, 512], BF16, tag="att", name=f"att{w}")
                    nc.vector.tensor_tensor(
                        att[:, 0:256], P["av"][:, 0:256], exsb[:, 0:256],
                        op=ALU.mult,
                    )
                    nc.vector.tensor_tensor(
                        att[0:64, 256:512], P["av"][0:64, 256:512],
                        exsb[0:64, 256:512], op=ALU.mult,
                    )

                # ---- 3. xres load; token-major v tiles via PE transpose ----
                if prev:
                    xres = a_sb.tile([128, 360], F32, tag="xres", name=f"xres{w}")
                    nc.sync.dma_start(
                        xres[:].rearrange("p (j d) -> p j d", d=C),
                        xrw_d[256 * (w - 1) : 256 * w, :].rearrange(
                            "(j p) d -> p j d", p=128
                        ),
                    )
                if cur:
                    kw0, kw1, vwT0, vwT1 = kvt_next
                    vwp = a_ep.tile([128, 960], BF16, tag="ep", name=f"vwp{w}")
                    vw = [
                        a_vw.tile([128, 192], BF16, tag=f"vw{i}", name=f"vw{w}_{i}")
                        for i in range(5)
                    ]
                    # full-128 slices go via DMA transpose (idle sync queue);
                    # the 64-wide chunk-4 slices + two vwT1 ones stay on PE
                    for i, off in enumerate(KOFF):
                        kc = KC[i]
                        pe_parts = []
                        if i < 4:
                            nc.sync.dma_start_transpose(
                                vw[i][:, 0:128], vwT0[:, off : off + 128]
                            )
                        else:
                            pe_parts.append((0, 128, vwT0, IDT[:]))
                        if i < 2:
                            nc.sync.dma_start_transpose(
                                vw[i][:, 128:192], vwT1[:, off : off + 128]
                            )
                        else:
                            pe_parts.append((128, 192, vwT1, IDT[0:64, 0:64]))
                        for lo, hi, src, idt in pe_parts:
                            nc.tensor.transpose(
                                vwp[0:kc, 192 * i + lo : 192 * i + hi],
                                src[:, off : off + kc],
                                idt,
                            )
                        if pe_parts:
                            lo = pe_parts[0][0]
                            nc.vector.tensor_copy(
                                vw[i][0:kc, lo:192],
                                vwp[0:kc, 192 * i + lo : 192 * i + 192],
                            )

                # ---- 3b. stage k^T/v^T window slices for window w+1 ----
                if w + 1 < NWIN:
                    nwrl, nwc = (w + 1) // 16, (w + 1) % 16
                    nr0, nc0 = WS * nwrl, WS * nwc
                    kw0n = a_kv.tile([128, 576], BF16, tag="kw0", name=f"kw0_{w+1}")
                    kw1n = a_kv.tile([64, 576], BF16, tag="kw1", name=f"kw1_{w+1}")
                    vwT0n = a_kv.tile([128, 576], BF16, tag="vwT0", name=f"vwT0_{w+1}")
                    vwT1n = a_kv.tile([64, 576], BF16, tag="vwT1", name=f"vwT1_{w+1}")
                    nc.vector.tensor_copy(
                        kw0n[:].rearrange("p (r c) -> p r c", c=OWS),
                        k03_pat[:, nr0 : nr0 + OWS, nc0 : nc0 + OWS],
                    )
                    nc.vector.tensor_copy(
                        kw1n[:].rearrange("p (r c) -> p r c", c=OWS),
                        qk45_pat[64:128, nr0 : nr0 + OWS, nc0 : nc0 + OWS],
                    )
                    nc.vector.tensor_copy(
                        vwT0n[:].rearrange("p (r c) -> p r c", c=OWS),
                        v03_pat[:, nr0 : nr0 + OWS, nc0 : nc0 + OWS],
                    )
                    nc.vector.tensor_copy(
                        vwT1n[:].rearrange("p (r c) -> p r c", c=OWS),
                        v45_pat[:, nr0 : nr0 + OWS, nc0 : nc0 + OWS],
                    )
                    kvt_next = (kw0n, kw1n, vwT0n, vwT1n)

                # ---- 5. S^T + exp + attn@V for window w ----
                if cur:
                    av = a_avp.tile([128, 512], F32, tag="av", name=f"av{w}")
                    es_list = []

                    def st_chunk(ch):
                        off, kc = KOFF[ch], KC[ch]
                        sta = a_sta.tile(
                            [128, 1024], F32, tag="sta", name=f"sta{w}_{ch}"
                        )
                        stb = a_stb.tile(
                            [128, 1024], F32, tag="stb", name=f"stb{w}_{ch}"
                        )
                        for wave in STW:
                            for h in wave:
                                if h < 4:
                                    ktile, qtile = kw0, q03_pat
                                    kr, qr = 32 * h, 32 * h
                                else:
                                    ktile, qtile = kw1, qk45_pat
                                    kr, qr = 32 * (h - 4), 32 * (h - 4)
                                st = sta if h in (0, 1, 4, 5) else stb
                                nc.tensor.matmul(
                                    st[0:kc, STC[h] : STC[h] + 256],
                                    ktile[kr : kr + 32, off : off + kc],
                                    qtile[
                                        qr : qr + 32,
                                        PADW + r0 : PADW + r0 + WS,
                                        PADW + c0 : PADW + c0 + WS,
                                    ],
                                    start=True, stop=True,
                                    tile_position=(TPR[h], 0),
                                )
                        return sta, stb

                    def exp_chunk(ch, sta, stb):
                        kc = KC[ch]
                        es = a_es.tile(
                            [128, 1536], BF16, tag="es", name=f"es{w}_{ch}"
                        )
                        nc.scalar.activation(
                            es[0:kc, 0:1024], sta[0:kc, :], AF.Exp
                        )
                        nc.scalar.activation(
                            es[0:kc, 1024:1536].rearrange(
                                "p (a b) -> p a b", b=256
                            ),
                            stb[0:kc, :].rearrange("p (a b) -> p a b", b=512)[
                                :, :, 0:256
                            ],
                            AF.Exp,
                        )
                        es_list.append(es)

                    def av_one(h, ro, co, ch):
                        # jobs with disjoint psum partitions may hold
                        # concurrently-open accumulation groups; jobs sharing
                        # partitions+bank (h0/h4, h1/h5) must not interleave
                        kc = KC[ch]
                        nc.tensor.matmul(
                            av[ro : ro + 32, co : co + 256],
                            vw[ch][0:kc, VC[h] : VC[h] + 32],
                            es_list[ch][0:kc, EC[h] : EC[h] + 256],
                            start=(ch == 0), stop=(ch == 4),
                            tile_position=(0, ro),
                            skip_group_check=True,
                        )

                    def av_chunk(ch):
                        for h, ro, co in AVJ[:4]:
                            av_one(h, ro, co, ch)

                    sts = [st_chunk(0)]
                    exp_chunk(0, *sts[0])
                    sts.append(st_chunk(1))
                    exp_chunk(1, *sts[1])
                    av_chunk(0)
                    sts.append(st_chunk(2))
                    exp_chunk(2, *sts[2])
                    av_chunk(1)
                    sts.append(st_chunk(3))
                    exp_chunk(3, *sts[3])
                    av_chunk(2)
                    sts.append(st_chunk(4))
                    exp_chunk(4, *sts[4])
                    av_chunk(3)
                    av_chunk(4)
                    for ch in range(5):
                        for h, ro, co in AVJ[4:]:
                            av_one(h, ro, co, ch)

                # ---- 6. projection + residual + store for window w-1 ----
                if prev:
                    P = st_prev
                    pp = a_ep.tile([128, 360], F32, tag="ep", name=f"pp{w}")
                    for qc in range(2):
                        nc.tensor.matmul(
                            pp[:, 180 * qc : 180 * qc + 180],
                            att[:, 128 * qc : 128 * (qc + 1)],
                            WP0[:],
                            start=True, stop=False,
                        )
                        nc.tensor.matmul(
                            pp[:, 180 * qc : 180 * qc + 180],
                            att[0:64, 256 + 128 * qc : 256 + 128 * (qc + 1)],
                            WP1[:],
                            start=False, stop=True,
                        )
                    ot = a_sb.tile([128, 360], F32, tag="ot", name=f"ot{w}", bufs=3)
                    nc.vector.tensor_tensor(ot[:], pp[:], xres[:], op=ALU.add)
                    st_prev["ot"] = ot

                # ---- 6b. store window w-2 (data long ready: no queue block)
                if 2 <= w <= NWIN + 1:
                    nc.sync.dma_start(
                        out_d[256 * (w - 2) : 256 * (w - 1), :].rearrange(
                            "(j p) d -> p j d", p=128
                        ),
                        wstate[w - 2]["ot"][:].rearrange("p (j d) -> p j d", d=C),
                    )

                # ---- debug dumps for window 0 ----
                if cur and w == 0 and kstage:
                    dbg = a_sb.tile([128, 2048], F32, tag="dbg", bufs=1)
                    if kstage == "q":
                        nc.vector.tensor_copy(dbg[:, 0:2048], Q03[:, 0:2048])
                    elif kstage == "kw":
                        nc.vector.tensor_copy(dbg[:, 0:576], kw0[:])
                        nc.vector.tensor_copy(dbg[0:64, 576:1152], kw1[:])
                    elif kstage == "vw0":
                        nc.vector.tensor_copy(dbg[:, 0:192], vw[0][:])
                        nc.vector.tensor_copy(dbg[:, 192:384], vw[4][:])
                    elif kstage == "es0":
                        nc.vector.tensor_copy(dbg[:, 0:1536], es_list[0][:])
                    elif kstage == "es4":
                        nc.vector.tensor_copy(dbg[0:64, 0:1536], es_list[4][0:64, :])
                    elif kstage == "av":
                        nc.vector.tensor_copy(dbg[:, 0:512], av[:])
                    nc.sync.dma_start(dbg_d[:, :], dbg[:])

                # ---- 7. rowsum snapshot for window w ----
                if cur:
                    rsb = a_sb.tile([128, 512], F32R, tag="rsb", name=f"rsb{w}")
                    nc.vector.tensor_copy(rsb[:], av[:])
                    wstate[w] = {"av": av, "rsb": rsb}
                    wstate.pop(w - 3, None)

    nc.compile()
    return nc


def _prep_host(inputs):
    x = np.ascontiguousarray(inputs["x"], dtype=np.float32)[0]  # [65536, 180]
    norm_w = np.asarray(inputs["norm_w"], np.float32)
    norm_b = np.asarray(inputs["norm_b"], np.float32)
    q_w = np.asarray(inputs["q_w"], np.float32)
    q_b = np.asarray(inputs["q_b"], np.float32)
    kv_w = np.asarray(inputs["kv_w"], np.float32)
    kv_b = np.asarray(inputs["kv_b"], np.float32)
    proj_w = np.asarray(inputs["proj_w"], np.float32)
    proj_b = np.asarray(inputs["proj_b"], np.float32)

    scale = HD ** -0.5
    Wq = norm_w[:, None] * q_w * scale
    bq = (norm_b @ q_w + q_b) * scale
    Wk = norm_w[:, None] * kv_w[:, :C]
    bk = norm_b @ kv_w[:, :C] + kv_b[:C]
    Wv = norm_w[:, None] * kv_w[:, C:]
    bv = norm_b @ kv_w[:, C:] + kv_b[C:]

    # wqkv [181, 576]: q03 | k03 | v03 | (q45|k45) | v45, 32-col head blocks;
    # v blocks carry 1.0 at row 180 in cols 30/31 (denominator ride-along)
    wqkv = np.zeros((181, 576), np.float32)

    def put(colbase, h, Wm, bm, ones):
        col = colbase + 32 * (h % 4)
        wqkv[:C, col : col + HD] = Wm[:, HD * h : HD * (h + 1)]
        wqkv[C, col : col + HD] = bm[HD * h : HD * (h + 1)]
        if ones:
            wqkv[C, col + 30] = 1.0
            wqkv[C, col + 31] = 1.0

    for h in range(4):
        put(0, h, Wq, bq, False)
        put(128, h, Wk, bk, False)
        put(256, h, Wv, bv, True)
    for h in (4, 5):
        put(384, h, Wq, bq, False)
        put(448, h, Wk, bk, False)
        put(512, h, Wv, bv, True)

    # wp [192, 180]: rows 0:128 = proj rows h0-3 (32-blocks), 128:192 h4-5
    wp = np.zeros((192, C), np.float32)
    for h in range(NH):
        row = 32 * h if h < 4 else 128 + 32 * (h - 4)
        wp[row : row + HD, :] = proj_w[HD * h : HD * (h + 1), :]

    ident = np.eye(128, dtype=bfnp)
    e128 = np.zeros((128, 128), np.float32)
    for j in range(4):
        e128[32 * j + 30, 32 * j : 32 * j + 32] = 1.0
    e64 = np.zeros((64, 64), np.float32)
    e64[30, 0:32] = 1.0
    e64[62, 32:64] = 1.0

    # per-core slabs
    xg = x.reshape(H, W, C)
    xpad = np.zeros((H + 2 * PADW, CS, C), np.float32)
    xpad[PADW : PADW + H, PADW : PADW + W, :] = xg
    xres_full = (x + proj_b).reshape(H, W, C)

    in_maps = []
    for c in range(NCORES):
        slab_x = np.zeros((T, C), np.float32)
        slab_x[:TS_REAL] = xpad[32 * c : 32 * c + RS].reshape(TS_REAL, C)
        # window-major residual: [2 wrl, 16 r, 16 wc, 16 cc, C] ->
        # (wrl, wc, r, cc)
        xr = xres_full[32 * c : 32 * c + 32].reshape(2, 16, 16, 16, C)
        xr = np.ascontiguousarray(xr.transpose(0, 2, 1, 3, 4)).reshape(8192, C)
        in_maps.append(
            {
                "xs": slab_x,
                "xrw": xr,
                "wqkv": wqkv.astype(bfnp),
                "wp": wp.astype(bfnp),
                "ident": ident,
                "e128": e128,
                "e64": e64,
            }
        )
    return in_maps


def kernel(**inputs):
    global _CACHED, LAST_RESULTS
    if _CACHED is None:
        _CACHED = _build_program()
    nc = _CACHED
    in_maps = _prep_host(inputs)
    res = run_bass_kernel_spmd(
        nc,
        in_maps,
        list(range(NCORES)),
        trace=bool(int(os.environ.get("KTRACE", "0"))),
    )
    LAST_RESULTS = res
    out = np.empty((1, H * W, C), np.float32)
    og = out[0].reshape(H, W, C)
    for c in range(NCORES):
        # un-permute window-major [2, 16 wc, 16 r, 16 cc, C] -> rows/cols
        o = res.results[c]["out"].reshape(2, 16, 16, 16, C)
        og[32 * c : 32 * c + 32] = o.transpose(0, 2, 1, 3, 4).reshape(32, W, C)
    return out
